# revision 33
# baseline (speedup 1.0000x reference)
"""Trainium2 Bass kernel for nn_DecentralizedCoordinator (GNN message passing).

Strategy (8 NeuronCores, SPMD), v5:
- Nodes degree-sorted, round-robin sharded: global degree rank r ->
  core r%8, block (r//8)//128, slot (r//8)%128. Blocks hold same-degree
  nodes, so the edge-source feature table ET needs exactly
  Kb = max-indeg-in-block identity-matmul columns per block: no one-hot
  tail region at all, ~6% padding.
- Launch 1 (DMA-bound): logits via per-block xfT-stationary matmuls
  (out [128 nodes, 1] per block into one PSUM bank), segment sums via
  identity-lhsT matmuls over the streamed ET, mean = sums * recip on DVE
  -> means (bf16, dst-major) + logits (f32 [128, BPC]) to DRAM.
- Host between launches (index-pattern reshuffles only): assemble
  means_full[node] table, ragged per-chunk padded election layout
  epad/srcp1 from logits.
- Launch 2 (GpSimd descriptor-gen bound): leader election (exact
  reference tie-break) on DVE, indirect-DMA gather of the LEADER'S MEAN
  row, then transpose + MLP (w1 -> gelu+b1 -> w2 -> +b2) on the gathered
  rows, pipelined per chunk so PE/Scalar hide under the gathers.

Host only shards/permutes/reshuffles by precomputed index patterns; every
operation on runtime values (logits, sums, means, MLP, comparisons,
election, the leader gather) is on device.
"""
import hashlib
import sys

import numpy as np
import ml_dtypes

sys.path.insert(0, "/opt/trn_rl_repo")

import concourse.bass as bass
import concourse.tile as tile
from concourse import bacc, mybir
from concourse.bass_utils import run_bass_kernel_spmd
from concourse.masks import make_identity

dt = mybir.dt
bf16 = ml_dtypes.bfloat16

P = 128
NCORES = 8
BPC = 98                 # dst blocks per core
NPC = BPC * P            # 12544 nodes per core
NPAD = NCORES * NPC      # 100352 padded node count
N_NODES = 100000
H = 128
C = 128
NEG = -3.0e38
CH = 16                  # blocks per launch-2 indirect chunk
NB_IND = 0               # blocks on the indirect-gather path (rest: masked)
MSW = 72                 # masked-window column budget

CORES = list(range(NCORES))


def _preprocess(edge_index):
    row = np.asarray(edge_index[0], np.int64)
    col = np.asarray(edge_index[1], np.int64)
    E = len(row)

    indeg = np.bincount(col, minlength=N_NODES)

    # degree-sorted round-robin assignment: rank r -> (r%8, (r//8)//128,
    # (r//8)%128)
    order = np.argsort(-indeg, kind="stable")          # rank -> node
    rr = np.arange(N_NODES)
    kk_of_rank = rr % NCORES
    pos = rr // NCORES
    bb_of_rank = pos // P
    pp_of_rank = pos % P
    node2kbp = np.zeros((N_NODES, 3), np.int64)
    node2kbp[order, 0] = kk_of_rank
    node2kbp[order, 1] = bb_of_rank
    node2kbp[order, 2] = pp_of_rank
    inv = np.full((NCORES, BPC, P), -1, np.int64)
    inv[kk_of_rank, bb_of_rank, pp_of_rank] = order

    # per-block column count (shared across cores): max indeg in the
    # 1024-rank group = indeg of its first (sorted desc)
    Kb = np.zeros(BPC, np.int64)
    for b in range(BPC):
        r0 = b * NCORES * P
        Kb[b] = indeg[order[r0]] if r0 < N_NODES else 0
    cb = np.concatenate([[0], np.cumsum(Kb)])          # col offset per block
    K = int(cb[-1])

    # edges grouped by dst, rank within dst
    dorder = np.argsort(col, kind="stable")
    row_d = row[dorder]
    col_d = col[dorder]
    dst_starts = np.concatenate([[0], np.cumsum(indeg)])
    ranks = np.arange(E) - dst_starts[col_d]

    kk = node2kbp[col_d, 0]
    bb = node2kbp[col_d, 1]
    ppos = node2kbp[col_d, 2]

    srcidx = np.full((NCORES, K * P), N_NODES, np.int64)
    srcidx[kk, (cb[bb] + ranks) * P + ppos] = row_d

    # recip of true in-degree per owned node, [NCORES, P, BPC]
    cnt = np.where(inv >= 0, indeg[np.maximum(inv, 0)], 0.0)   # [NC,BPC,P]
    recip = np.ascontiguousarray(
        (1.0 / np.maximum(cnt, 1.0)).transpose(0, 2, 1)).astype(np.float32)

    # ET stream windows: whole blocks, ~56 cols each
    windows = []           # (b_start, b_end, col0, ncols)
    b = 0
    while b < BPC:
        b1 = b
        ncols = 0
        while b1 < BPC and (ncols + Kb[b1] <= 56 or b1 == b):
            ncols += int(Kb[b1])
            b1 += 1
        windows.append((b, b1, int(cb[b]), ncols))
        b = b1

    # ragged election layout (indirect blocks only): chunk c covers CH
    # blocks, width Wc = max ext-degree (indeg+1) in chunk
    chunks = []            # (b0, ngb, Wc, off)
    off = 0
    for b0 in range(0, NB_IND, CH):
        ngb = min(CH, NB_IND - b0)
        r0 = b0 * NCORES * P
        Wc = int(indeg[order[r0]]) + 1 if r0 < N_NODES else 1
        chunks.append((b0, ngb, Wc, off))
        off += ngb * Wc
    SWR = off

    WMAX = max([w for (_, _, w, _) in chunks], default=1)
    elog_src = np.full((NCORES, P, SWR), -1, np.int64)
    for (b0, ngb, Wc, coff) in chunks:
        for bi in range(ngb):
            b = b0 + bi
            base = coff + bi * Wc
            for k in range(NCORES):
                for p in range(P):
                    d = int(inv[k, b, p])
                    if d < 0:
                        continue
                    s0, s1 = int(dst_starts[d]), int(dst_starts[d + 1])
                    m = s1 - s0
                    elog_src[k, p, base] = d
                    if m > 0:
                        elog_src[k, p, base + 1: base + 1 + m] = row_d[s0:s1]
    srcp1 = np.where(elog_src >= 0, elog_src + 1, 0).astype(np.float32)

    # ---- masked-sum side (blocks NB_IND..BPC) -------------------------
    # per-dst dedup candidate list: [self] + unique in-edge srcs != self
    srt = np.lexsort((row_d, col_d))
    c2, r2 = col_d[srt], row_d[srt]
    uniq = np.ones(E, bool)
    uniq[1:] = (c2[1:] != c2[:-1]) | (r2[1:] != r2[:-1])
    uniq &= (r2 != c2)
    cu, ru = c2[uniq], r2[uniq]
    dl = np.bincount(cu, minlength=N_NODES)            # dedup in-deg (no self)
    du_starts = np.concatenate([[0], np.cumsum(dl)])

    # per-block candidate width (shared across cores)
    wd = np.zeros(BPC, np.int64)
    for b in range(NB_IND, BPC):
        r0, r1 = b * NCORES * P, min((b + 1) * NCORES * P, N_NODES)
        wd[b] = (dl[order[r0:r1]].max() + 1) if r1 > r0 else 1

    # masked windows: blocks with uniform padded width Kw, nblk*Kw <= MSW
    mwin = []              # (b0, nblk, Kw, col0)
    b = NB_IND
    mc = 0
    while b < BPC:
        b1 = b
        Kw = int(wd[b])
        while b1 < BPC:
            nKw = max(Kw, int(wd[b1]))
            if ((b1 - b + 1) * nKw > MSW or b1 - b >= 12) and b1 > b:
                break
            Kw = nKw
            b1 += 1
        mwin.append((b, b1 - b, Kw, mc))
        mc += (b1 - b) * Kw
        b = b1
    MSCOLS = mc

    msrc = np.full((NCORES, P, MSCOLS), N_NODES, np.int64)
    for (b0m, nblk, Kw, col0m) in mwin:
        for bi in range(nblk):
            b = b0m + bi
            base = col0m + bi * Kw
            for k in range(NCORES):
                for p in range(P):
                    d = int(inv[k, b, p])
                    if d < 0:
                        continue
                    msrc[k, p, base] = d
                    s0, s1 = int(du_starts[d]), int(du_starts[d + 1])
                    m = s1 - s0
                    if m > 0:
                        msrc[k, p, base + 1: base + 1 + m] = ru[s0:s1]

    return dict(
        Kb=Kb, cb=cb, K=K, windows=windows,
        srcidx=srcidx, recip=recip,
        chunks=chunks, SWR=SWR, WMAX=WMAX, elog_src=elog_src, srcp1=srcp1,
        mwin=mwin, MSCOLS=MSCOLS, msrc=msrc,
        node2kbp=node2kbp, inv=inv,
    )


# ---------------------------------------------------------------------------
# launch 1: logits + segment sums + mean
# ---------------------------------------------------------------------------

def _build_l1(pp):
    Kb = pp["Kb"]
    cb = pp["cb"]
    K = pp["K"]
    windows = pp["windows"]

    nc = bacc.Bacc("TRN2", target_bir_lowering=False, debug=False,
                   num_devices=NCORES)
    et_d = nc.dram_tensor("et", [P, K * H], dt.bfloat16,
                          kind="ExternalInput")
    xfT_d = nc.dram_tensor("xfT", [P, NPC], dt.float32,
                           kind="ExternalInput")
    recip_d = nc.dram_tensor("recip", [P, BPC], dt.float32,
                             kind="ExternalInput")
    wrepc_d = nc.dram_tensor("wrepc", [H, 1], dt.float32,
                             kind="ExternalInput")
    blead_d = nc.dram_tensor("blead", [1, 1], dt.float32,
                             kind="ExternalInput")

    logits_o = nc.dram_tensor("logits_o", [1, NPC], dt.float32,
                              kind="ExternalOutput")
    means_o = nc.dram_tensor("means_o", [P, BPC * H], dt.bfloat16,
                             kind="ExternalOutput")

    SB = 14                                        # blocks per means stage

    with tile.TileContext(nc) as tc:
        with (
            tc.tile_pool(name="const", bufs=1) as cp,
            tc.tile_pool(name="g", bufs=4) as gp,
            tc.tile_pool(name="stage", bufs=2) as stp,
            tc.tile_pool(name="sums_ps", bufs=4, space="PSUM") as sums_pp,
            tc.tile_pool(name="lg_ps", bufs=2, space="PSUM") as lg_pp,
        ):
            recip_t = cp.tile([P, BPC], dt.float32)
            nc.sync.dma_start(recip_t[:], recip_d[:, :])
            wrepc_t = cp.tile([H, 1], dt.float32)
            nc.sync.dma_start(wrepc_t[:], wrepc_d[:, :])
            blead_t = cp.tile([1, 1], dt.float32)
            nc.sync.dma_start(blead_t[:], blead_d[:, :])
            xfT_t = cp.tile([P, NPC], dt.float32)
            logits_sb = cp.tile([1, NPC], dt.float32)
            ident_f = cp.tile([P, P], dt.bfloat16)
            make_identity(nc, ident_f[:])

            stage_out = None
            for (bw0, bw1, col0, ncols) in windows:
                G = gp.tile([P, 56 * H], dt.bfloat16, tag="g")
                if ncols > 0:
                    nc.sync.dma_start(G[:, : ncols * H],
                                      et_d[:, col0 * H: (col0 + ncols) * H])
                q0w, q1w = bw0 * P, bw1 * P
                nc.sync.dma_start(xfT_t[:, q0w: q1w], xfT_d[:, q0w: q1w])
                for b in range(bw0, bw1):
                    sj = b % SB
                    if sj == 0:
                        stage_out = stp.tile([P, SB * H], dt.bfloat16,
                                             tag="st")
                    nb = int(Kb[b])
                    c0 = int(cb[b]) - col0
                    sums_ps = sums_pp.tile([P, H], dt.float32, space="PSUM",
                                           tag="sums")
                    if nb == 0:
                        nc.vector.memset(sums_ps[:], 0.0)
                    for j in range(nb):
                        nc.tensor.matmul(
                            out=sums_ps[:], lhsT=ident_f[:],
                            rhs=G[:, (c0 + j) * H: (c0 + j + 1) * H],
                            start=(j == 0), stop=(j == nb - 1))
                    nc.vector.tensor_scalar_mul(
                        stage_out[:, sj * H: (sj + 1) * H], sums_ps[:],
                        recip_t[:, b: b + 1])
                    if sj == SB - 1 or b == BPC - 1:
                        b0s = b - sj
                        nc.sync.dma_start(
                            means_o[:, b0s * H: (b + 1) * H],
                            stage_out[:, : (sj + 1) * H])
                # logits for this window's nodes: w_lead-stationary chunks
                for q0 in range(q0w, q1w, 448):
                    nq = min(448, q1w - q0)
                    lg_ps = lg_pp.tile([1, 448], dt.float32, space="PSUM",
                                       tag="lg")
                    nc.tensor.matmul(out=lg_ps[:, :nq], lhsT=wrepc_t[:],
                                     rhs=xfT_t[:, q0: q0 + nq],
                                     start=True, stop=True)
                    nc.scalar.activation(
                        logits_sb[:, q0: q0 + nq], lg_ps[:, :nq],
                        mybir.ActivationFunctionType.Identity,
                        bias=blead_t[:, :1])
            nc.sync.dma_start(logits_o[:, :], logits_sb[:])
    nc.compile()
    return nc


# ---------------------------------------------------------------------------
# launch 2: leader election + mean gather + MLP
# ---------------------------------------------------------------------------

def _view3(t, ngb, wc):
    """[P, ngb, wc] strided view of a [P, >=ngb*wc] tile."""
    a = t[:]
    return bass.AP(a.tensor, a.offset, [a.ap[0], [wc, ngb], [1, wc]])


def _build_l2(pp):
    chunks = pp["chunks"]
    SWR = pp["SWR"]
    WMAX = pp["WMAX"]
    mwin = pp["mwin"]
    MSCOLS = pp["MSCOLS"]

    nc = bacc.Bacc("TRN2", target_bir_lowering=False, debug=False,
                   num_devices=NCORES)
    SWR1 = max(SWR, 1)
    ep_d = nc.dram_tensor("epad", [P, SWR1], dt.float32,
                          kind="ExternalInput")
    sp1_d = nc.dram_tensor("srcp1", [P, SWR1], dt.float32,
                           kind="ExternalInput")
    means_d = nc.dram_tensor("meansfull", [NPAD, H], dt.bfloat16,
                             kind="ExternalInput")
    melog_d = nc.dram_tensor("melog", [P, MSCOLS], dt.float32,
                             kind="ExternalInput")
    mset_d = nc.dram_tensor("mset", [P, MSCOLS * H], dt.bfloat16,
                            kind="ExternalInput")
    w1_d = nc.dram_tensor("w1", [H, H], dt.bfloat16, kind="ExternalInput")
    b1_d = nc.dram_tensor("b1", [P, 1], dt.float32, kind="ExternalInput")
    w2_d = nc.dram_tensor("w2", [H, C], dt.bfloat16, kind="ExternalInput")
    b2_d = nc.dram_tensor("b2c", [C, 1], dt.float32, kind="ExternalInput")
    out_o = nc.dram_tensor("out_o", [C, NPC], dt.bfloat16,
                           kind="ExternalOutput")

    MB = 4                                          # blocks per MLP group

    with tile.TileContext(nc) as tc:
        with (
            tc.tile_pool(name="const", bufs=1) as cp,
            tc.tile_pool(name="ein", bufs=2) as eip,
            tc.tile_pool(name="ework", bufs=2) as ewp,
            tc.tile_pool(name="lead", bufs=2) as ldp,
            tc.tile_pool(name="rows", bufs=2) as rp_,
            tc.tile_pool(name="win", bufs=len(mwin)) as wnp,
            tc.tile_pool(name="gms", bufs=4) as gmp,
            tc.tile_pool(name="mw", bufs=3) as mwp,
            tc.tile_pool(name="mst", bufs=2) as mp,
            tc.tile_pool(name="ost", bufs=2) as stp,
            tc.tile_pool(name="ostm", bufs=2) as stp2,
            tc.tile_pool(name="tr_ps", bufs=2, space="PSUM") as tr_pp,
            tc.tile_pool(name="sel_ps", bufs=3, space="PSUM") as sel_pp,
            tc.tile_pool(name="mlp_ps", bufs=2, space="PSUM") as mlp_pp,
        ):
            w1_t = cp.tile([H, H], dt.bfloat16)
            nc.scalar.dma_start(w1_t[:], w1_d[:, :])
            b1_t = cp.tile([P, 1], dt.float32)
            nc.scalar.dma_start(b1_t[:], b1_d[:, :])
            w2_t = cp.tile([H, C], dt.bfloat16)
            nc.scalar.dma_start(w2_t[:], w2_d[:, :])
            b2_t = cp.tile([C, 1], dt.float32)
            nc.scalar.dma_start(b2_t[:], b2_d[:, :])
            ident_f = cp.tile([P, P], dt.bfloat16)
            make_identity(nc, ident_f[:])
            melog_t = cp.tile([P, MSCOLS], dt.float32)
            nc.scalar.dma_start(melog_t[:], melog_d[:, :])

            def emit_mlp(meanT_stage, nmb, stage_out, g0):
                hpre_ps = mlp_pp.tile([P, MB * H], dt.float32,
                                      space="PSUM", tag="mlp")
                nc.tensor.matmul(out=hpre_ps[:, : nmb * H], lhsT=w1_t[:],
                                 rhs=meanT_stage[:, : nmb * P],
                                 start=True, stop=True)
                hT_stage = mp.tile([P, MB * H], dt.bfloat16, tag="hT")
                nc.scalar.activation(hT_stage[:, : nmb * H],
                                     hpre_ps[:, : nmb * H],
                                     mybir.ActivationFunctionType.Gelu,
                                     bias=b1_t[:, :1])
                rep_ps = mlp_pp.tile([P, MB * P], dt.float32,
                                     space="PSUM", tag="mlp")
                nc.tensor.matmul(out=rep_ps[:, : nmb * P], lhsT=w2_t[:],
                                 rhs=hT_stage[:, : nmb * H],
                                 start=True, stop=True)
                nc.scalar.activation(
                    stage_out[:, g0 * P: (g0 + nmb) * P],
                    rep_ps[:, : nmb * P],
                    mybir.ActivationFunctionType.Identity,
                    bias=b2_t[:, :1])

            # phase 1: indirect-side elections (DVE) -> per-chunk leadi
            leadis = []
            for (b0, ngb, Wc, coff) in chunks:
                n = ngb * Wc
                ep = eip.tile([P, CH * WMAX], dt.float32, tag="ep")
                nc.sync.dma_start(ep[:, :n], ep_d[:, coff: coff + n])
                epv = _view3(ep, ngb, Wc)
                sp1 = eip.tile([P, CH * WMAX], dt.float32, tag="sp1")
                nc.sync.dma_start(sp1[:, :n], sp1_d[:, coff: coff + n])
                sp1v = _view3(sp1, ngb, Wc)

                sm = ewp.tile([P, CH], dt.float32, tag="sm")
                nc.vector.reduce_max(out=sm[:, :ngb], in_=epv,
                                     axis=mybir.AxisListType.X)
                mask = ewp.tile([P, CH * WMAX], dt.float32, tag="mask")
                maskv = _view3(mask, ngb, Wc)
                a = sm[:]
                sm_b = bass.AP(a.tensor, a.offset,
                               [a.ap[0], [1, ngb], [0, Wc]])
                nc.vector.tensor_tensor(out=maskv, in0=epv, in1=sm_b,
                                        op=mybir.AluOpType.is_equal)
                cand = ewp.tile([P, CH * WMAX], dt.float32, tag="cand")
                candv = _view3(cand, ngb, Wc)
                nc.vector.tensor_tensor(out=candv, in0=maskv, in1=sp1v,
                                        op=mybir.AluOpType.mult)
                lp1 = ewp.tile([P, CH], dt.float32, tag="lp1")
                nc.vector.reduce_max(out=lp1[:, :ngb], in_=candv,
                                     axis=mybir.AxisListType.X)
                leadf = ewp.tile([P, CH], dt.float32, tag="leadf")
                nc.vector.tensor_scalar(
                    out=leadf[:, :ngb], in0=lp1[:, :ngb], scalar1=-1.0,
                    scalar2=0.0,
                    op0=mybir.AluOpType.add, op1=mybir.AluOpType.max)
                leadi = ldp.tile([P, CH], dt.int32, tag="leadi")
                nc.vector.tensor_copy(leadi[:, :ngb], leadf[:, :ngb])
                leadis.append(leadi)

            # phase 2: issue all indirect gathers (GpSimd self-paces; they
            # drain after the mset stream quiesces)
            rows_l = []
            for ci, (b0, ngb, Wc, coff) in enumerate(chunks):
                leadi = leadis[ci]
                rows = rp_.tile([P, CH, H], dt.bfloat16, tag="rows")
                for j in range(ngb):
                    nc.gpsimd.indirect_dma_start(
                        out=rows[:, j, :],
                        out_offset=None,
                        in_=means_d[:, :],
                        in_offset=bass.IndirectOffsetOnAxis(
                            ap=leadi[:, j: j + 1], axis=0),
                    )
                rows_l.append(rows)

            # phase 3: masked-side winner masks (DVE)
            wins = []
            for (b0m, nblk, Kw, col0m) in mwin:
                nmc = nblk * Kw
                smx = ewp.tile([P, MSW], dt.float32, tag="smx")
                ml = melog_t[:, col0m: col0m + nmc]
                lv = bass.AP(ml.tensor, ml.offset,
                             [ml.ap[0], [Kw, nblk], [1, Kw]])
                nc.vector.reduce_max(out=smx[:, :nblk], in_=lv,
                                     axis=mybir.AxisListType.X)
                win = wnp.tile([P, MSW], dt.bfloat16, tag="win")
                winv = _view3(win, nblk, Kw)
                a = smx[:]
                smx_b = bass.AP(a.tensor, a.offset,
                                [a.ap[0], [1, nblk], [0, Kw]])
                nc.vector.tensor_tensor(out=winv, in0=lv, in1=smx_b,
                                        op=mybir.AluOpType.is_equal)
                wins.append(win)

            # phase 4: mset stream + select + MLP
            gp_elems, dve_elems = 0, 1
            for wi, (b0m, nblk, Kw, col0m) in enumerate(mwin):
                nmc = nblk * Kw
                G = gmp.tile([P, MSW * H], dt.bfloat16, tag="gms")
                nc.sync.dma_start(G[:, : nmc * H],
                                  mset_d[:, col0m * H: (col0m + nmc) * H])
                win = wins[wi]
                mw = mwp.tile([P, MSW * H], dt.bfloat16, tag="mw")
                ga = G[:, : nmc * H]
                gv = bass.AP(ga.tensor, ga.offset,
                             [ga.ap[0], [H, nmc], [1, H]])
                wa = win[:, : nmc]
                wv = bass.AP(wa.tensor, wa.offset,
                             [wa.ap[0], [1, nmc], [0, H]])
                ma = mw[:, : nmc * H]
                mv = bass.AP(ma.tensor, ma.offset,
                             [ma.ap[0], [H, nmc], [1, H]])
                if gp_elems * 2 < dve_elems:
                    eng = nc.gpsimd
                    gp_elems += nmc
                else:
                    eng = nc.vector
                    dve_elems += nmc
                eng.tensor_tensor(out=mv, in0=gv, in1=wv,
                                  op=mybir.AluOpType.mult)

                stage_out = stp2.tile([P, 12 * P], dt.bfloat16, tag="stm")
                meanT_stage = None
                for bi in range(nblk):
                    gj = bi % MB
                    if gj == 0:
                        meanT_stage = mp.tile([P, MB * P], dt.bfloat16,
                                              tag="meanT")
                    sel_ps = sel_pp.tile([P, 512], dt.float32, space="PSUM",
                                         tag="sel")
                    for j in range(Kw):
                        c = bi * Kw + j
                        nc.tensor.matmul(out=sel_ps[:, :P],
                                         lhsT=mw[:, c * H: (c + 1) * H],
                                         rhs=ident_f[:],
                                         start=(j == 0), stop=(j == Kw - 1))
                    nc.scalar.activation(
                        meanT_stage[:, gj * P: (gj + 1) * P], sel_ps[:, :P],
                        mybir.ActivationFunctionType.Copy)
                    if gj == MB - 1 or bi == nblk - 1:
                        emit_mlp(meanT_stage, gj + 1, stage_out, bi - gj)
                nc.scalar.dma_start(out_o[:, b0m * P: (b0m + nblk) * P],
                                    stage_out[:, : nblk * P])

            # phase 5: indirect-side MLP
            for ci, (b0, ngb, Wc, coff) in enumerate(chunks):
                rows = rows_l[ci]
                stage_out = stp.tile([P, CH * P], dt.bfloat16, tag="st")
                for g0 in range(0, ngb, MB):
                    nmb = min(MB, ngb - g0)
                    meanT_stage = mp.tile([P, MB * P], dt.bfloat16,
                                          tag="meanT")
                    for j in range(nmb):
                        meanT_ps = tr_pp.tile([P, P], dt.bfloat16,
                                              space="PSUM", tag="tr")
                        nc.tensor.transpose(meanT_ps[:], rows[:, g0 + j, :],
                                            ident_f[:])
                        nc.scalar.activation(
                            meanT_stage[:, j * P: (j + 1) * P], meanT_ps[:],
                            mybir.ActivationFunctionType.Copy)
                    emit_mlp(meanT_stage, nmb, stage_out, g0)
                nc.scalar.dma_start(out_o[:, b0 * P: (b0 + ngb) * P],
                                    stage_out[:, : ngb * P])
    nc.compile()
    return nc


# ---------------------------------------------------------------------------

_CACHE = {}


def _get(key, fn):
    if key not in _CACHE:
        _CACHE[key] = fn()
    return _CACHE[key]


def kernel(x, edge_index, w_lead, b_lead, w1, b1, w2, b2):
    x = np.asarray(x, np.float32)
    assert x.shape == (N_NODES, H)

    ekey = hashlib.md5(np.asarray(edge_index).tobytes()).hexdigest()
    pp = _get(("pp", ekey), lambda: _preprocess(edge_index))

    K = pp["K"]
    inv = pp["inv"]
    inv_flat = inv.reshape(NCORES, NPC)

    xbf_ext = np.zeros((N_NODES + 1, H), bf16)
    xbf_ext[:N_NODES] = x.astype(bf16)
    xpad = np.zeros((NPAD, H), np.float32)
    xpad[:N_NODES] = x

    wrepc = np.ascontiguousarray(
        np.asarray(w_lead, np.float32).reshape(H, 1))
    blead = np.full((1, 1), np.float32(b_lead), np.float32)
    w1f = np.ascontiguousarray(np.asarray(w1, np.float32).astype(bf16))
    b1c = np.ascontiguousarray(np.asarray(b1, np.float32).reshape(H, 1))
    w2f = np.ascontiguousarray(np.asarray(w2, np.float32).astype(bf16))
    b2c = np.ascontiguousarray(np.asarray(b2, np.float32).reshape(C, 1))

    # ---- launch 1 ----------------------------------------------------------
    nc1 = _get(("l1", ekey), lambda: _build_l1(pp))
    in_maps = []
    for k in range(NCORES):
        # ET[k]: [P, K*H] partition-major slot table (slot = c*128+p)
        et = xbf_ext[pp["srcidx"][k]].reshape(K, P, H).transpose(1, 0, 2)
        et = np.ascontiguousarray(et).reshape(P, K * H)
        # xfT: [H, NPC] owned-node features transposed (f32, for logits)
        xf = np.where((inv[k] >= 0)[:, :, None],
                      xpad[np.maximum(inv[k], 0)], 0.0)  # [BPC, P, H]
        xfT = np.ascontiguousarray(
            xf.reshape(NPC, H).T.astype(np.float32))
        in_maps.append({
            "et": et,
            "xfT": xfT,
            "recip": pp["recip"][k],
            "wrepc": wrepc,
            "blead": blead,
        })
    r1 = run_bass_kernel_spmd(nc1, in_maps, core_ids=CORES)

    logits_full = np.zeros(NPAD, np.float32)
    means_full = np.zeros((NPAD, H), bf16)
    for k in range(NCORES):
        lg = r1.results[k]["logits_o"].reshape(NPC)     # [b*128+p]
        mn = r1.results[k]["means_o"].reshape(P, BPC, H)
        mf = inv_flat[k] >= 0
        ids = inv_flat[k][mf]
        logits_full[ids] = lg[mf]
        m = inv[k] >= 0                                 # [BPC, P]
        means_full[inv[k][m]] = mn.transpose(1, 0, 2)[m]

    # ---- launch 2: election + mean gather + MLP ----------------------------
    nc2 = _get(("l2", ekey), lambda: _build_l2(pp))
    es = pp["elog_src"]
    msrc = pp["msrc"]
    MSCOLS = pp["MSCOLS"]
    logits_ext = np.full(NPAD + 1, NEG, np.float32)
    logits_ext[:NPAD] = logits_full
    in_maps2 = []
    zpad = np.zeros((P, 1), np.float32)
    for k in range(NCORES):
        if pp["SWR"] > 0:
            ep = np.ascontiguousarray(
                np.where(es[k] >= 0, logits_full[np.maximum(es[k], 0)],
                         NEG).astype(np.float32))
            sp1k = pp["srcp1"][k]
        else:
            ep, sp1k = zpad, zpad
        melog = np.where(msrc[k] < N_NODES, logits_ext[msrc[k]],
                         NEG).astype(np.float32)
        mset = means_full[msrc[k]].reshape(P, MSCOLS * H)
        in_maps2.append({
            "epad": ep,
            "srcp1": sp1k,
            "meansfull": means_full,
            "melog": np.ascontiguousarray(melog),
            "mset": np.ascontiguousarray(mset),
            "w1": w1f,
            "b1": b1c,
            "w2": w2f,
            "b2c": b2c,
        })
    r2 = run_bass_kernel_spmd(nc2, in_maps2, core_ids=CORES)

    out = np.zeros((N_NODES, C), np.float32)
    for k in range(NCORES):
        g = r2.results[k]["out_o"].reshape(C, BPC, P).astype(np.float32)
        node_rows = g.transpose(1, 2, 0).reshape(NPC, C)
        m = inv_flat[k] >= 0
        out[inv_flat[k][m]] = node_rows[m]
    return out


# revision 36
# speedup vs baseline: 1.1523x; 1.1523x over previous
"""Trainium2 Bass kernel for nn_DecentralizedCoordinator (GNN message passing).

Strategy (8 NeuronCores, SPMD), v5:
- Nodes degree-sorted, round-robin sharded: global degree rank r ->
  core r%8, block (r//8)//128, slot (r//8)%128. Blocks hold same-degree
  nodes, so the edge-source feature table ET needs exactly
  Kb = max-indeg-in-block identity-matmul columns per block: no one-hot
  tail region at all, ~6% padding.
- Launch 1 (DMA-bound): logits via per-block xfT-stationary matmuls
  (out [128 nodes, 1] per block into one PSUM bank), segment sums via
  identity-lhsT matmuls over the streamed ET, mean = sums * recip on DVE
  -> means (bf16, dst-major) + logits (f32 [128, BPC]) to DRAM.
- Host between launches (index-pattern reshuffles only): assemble
  means_full[node] table, ragged per-chunk padded election layout
  epad/srcp1 from logits.
- Launch 2 (GpSimd descriptor-gen bound): leader election (exact
  reference tie-break) on DVE, indirect-DMA gather of the LEADER'S MEAN
  row, then transpose + MLP (w1 -> gelu+b1 -> w2 -> +b2) on the gathered
  rows, pipelined per chunk so PE/Scalar hide under the gathers.

Host only shards/permutes/reshuffles by precomputed index patterns; every
operation on runtime values (logits, sums, means, MLP, comparisons,
election, the leader gather) is on device.
"""
import hashlib
import sys

import numpy as np
import ml_dtypes

sys.path.insert(0, "/opt/trn_rl_repo")

import concourse.bass as bass
import concourse.tile as tile
from concourse import bacc, mybir
from concourse.bass_utils import run_bass_kernel_spmd
from concourse.masks import make_identity

dt = mybir.dt
bf16 = ml_dtypes.bfloat16

P = 128
NCORES = 8
BPC = 98                 # dst blocks per core
NPC = BPC * P            # 12544 nodes per core
NPAD = NCORES * NPC      # 100352 padded node count
N_NODES = 100000
H = 128
C = 128
NEG = -3.0e38
CH = 16                  # blocks per launch-2 indirect chunk
NB_IND = 98              # blocks on the indirect-gather path (rest: masked)
MSW = 72                 # masked-window column budget

CORES = list(range(NCORES))


def _preprocess(edge_index):
    row = np.asarray(edge_index[0], np.int64)
    col = np.asarray(edge_index[1], np.int64)
    E = len(row)

    indeg = np.bincount(col, minlength=N_NODES)

    # degree-sorted round-robin assignment: rank r -> (r%8, (r//8)//128,
    # (r//8)%128)
    order = np.argsort(-indeg, kind="stable")          # rank -> node
    rr = np.arange(N_NODES)
    kk_of_rank = rr % NCORES
    pos = rr // NCORES
    bb_of_rank = pos // P
    pp_of_rank = pos % P
    node2kbp = np.zeros((N_NODES, 3), np.int64)
    node2kbp[order, 0] = kk_of_rank
    node2kbp[order, 1] = bb_of_rank
    node2kbp[order, 2] = pp_of_rank
    inv = np.full((NCORES, BPC, P), -1, np.int64)
    inv[kk_of_rank, bb_of_rank, pp_of_rank] = order

    # per-block column count (shared across cores): max indeg in the
    # 1024-rank group = indeg of its first (sorted desc)
    Kb = np.zeros(BPC, np.int64)
    for b in range(BPC):
        r0 = b * NCORES * P
        Kb[b] = indeg[order[r0]] if r0 < N_NODES else 0
    cb = np.concatenate([[0], np.cumsum(Kb)])          # col offset per block
    K = int(cb[-1])

    # edges grouped by dst, rank within dst
    dorder = np.argsort(col, kind="stable")
    row_d = row[dorder]
    col_d = col[dorder]
    dst_starts = np.concatenate([[0], np.cumsum(indeg)])
    ranks = np.arange(E) - dst_starts[col_d]

    kk = node2kbp[col_d, 0]
    bb = node2kbp[col_d, 1]
    ppos = node2kbp[col_d, 2]

    srcidx = np.full((NCORES, K * P), N_NODES, np.int64)
    srcidx[kk, (cb[bb] + ranks) * P + ppos] = row_d

    # recip of true in-degree per owned node, [NCORES, P, BPC]
    cnt = np.where(inv >= 0, indeg[np.maximum(inv, 0)], 0.0)   # [NC,BPC,P]
    recip = np.ascontiguousarray(
        (1.0 / np.maximum(cnt, 1.0)).transpose(0, 2, 1)).astype(np.float32)

    # ET stream windows: whole blocks, ~56 cols each
    windows = []           # (b_start, b_end, col0, ncols)
    b = 0
    while b < BPC:
        b1 = b
        ncols = 0
        while b1 < BPC and (ncols + Kb[b1] <= 56 or b1 == b):
            ncols += int(Kb[b1])
            b1 += 1
        windows.append((b, b1, int(cb[b]), ncols))
        b = b1

    # ragged election layout (indirect blocks only): chunk c covers CH
    # blocks, width Wc = max ext-degree (indeg+1) in chunk
    chunks = []            # (b0, ngb, Wc, off)
    off = 0
    for b0 in range(0, NB_IND, CH):
        ngb = min(CH, NB_IND - b0)
        r0 = b0 * NCORES * P
        Wc = int(indeg[order[r0]]) + 1 if r0 < N_NODES else 1
        chunks.append((b0, ngb, Wc, off))
        off += ngb * Wc
    SWR = off

    WMAX = max([w for (_, _, w, _) in chunks], default=1)
    elog_src = np.full((NCORES, P, SWR), -1, np.int64)
    for (b0, ngb, Wc, coff) in chunks:
        for bi in range(ngb):
            b = b0 + bi
            base = coff + bi * Wc
            for k in range(NCORES):
                for p in range(P):
                    d = int(inv[k, b, p])
                    if d < 0:
                        continue
                    s0, s1 = int(dst_starts[d]), int(dst_starts[d + 1])
                    m = s1 - s0
                    elog_src[k, p, base] = d
                    if m > 0:
                        elog_src[k, p, base + 1: base + 1 + m] = row_d[s0:s1]
    srcp1 = np.where(elog_src >= 0, elog_src + 1, 0).astype(np.float32)

    # ---- masked-sum side (blocks NB_IND..BPC) -------------------------
    # per-dst dedup candidate list: [self] + unique in-edge srcs != self
    srt = np.lexsort((row_d, col_d))
    c2, r2 = col_d[srt], row_d[srt]
    uniq = np.ones(E, bool)
    uniq[1:] = (c2[1:] != c2[:-1]) | (r2[1:] != r2[:-1])
    uniq &= (r2 != c2)
    cu, ru = c2[uniq], r2[uniq]
    dl = np.bincount(cu, minlength=N_NODES)            # dedup in-deg (no self)
    du_starts = np.concatenate([[0], np.cumsum(dl)])

    # per-block candidate width (shared across cores)
    wd = np.zeros(BPC, np.int64)
    for b in range(NB_IND, BPC):
        r0, r1 = b * NCORES * P, min((b + 1) * NCORES * P, N_NODES)
        wd[b] = (dl[order[r0:r1]].max() + 1) if r1 > r0 else 1

    # masked windows: blocks with uniform padded width Kw, nblk*Kw <= MSW
    mwin = []              # (b0, nblk, Kw, col0)
    b = NB_IND
    mc = 0
    while b < BPC:
        b1 = b
        Kw = int(wd[b])
        while b1 < BPC:
            nKw = max(Kw, int(wd[b1]))
            if ((b1 - b + 1) * nKw > MSW or b1 - b >= 12) and b1 > b:
                break
            Kw = nKw
            b1 += 1
        mwin.append((b, b1 - b, Kw, mc))
        mc += (b1 - b) * Kw
        b = b1
    MSCOLS = mc

    msrc = np.full((NCORES, P, MSCOLS), N_NODES, np.int64)
    for (b0m, nblk, Kw, col0m) in mwin:
        for bi in range(nblk):
            b = b0m + bi
            base = col0m + bi * Kw
            for k in range(NCORES):
                for p in range(P):
                    d = int(inv[k, b, p])
                    if d < 0:
                        continue
                    msrc[k, p, base] = d
                    s0, s1 = int(du_starts[d]), int(du_starts[d + 1])
                    m = s1 - s0
                    if m > 0:
                        msrc[k, p, base + 1: base + 1 + m] = ru[s0:s1]

    return dict(
        Kb=Kb, cb=cb, K=K, windows=windows,
        srcidx=srcidx, recip=recip,
        chunks=chunks, SWR=SWR, WMAX=WMAX, elog_src=elog_src, srcp1=srcp1,
        mwin=mwin, MSCOLS=MSCOLS, msrc=msrc,
        node2kbp=node2kbp, inv=inv,
    )


# ---------------------------------------------------------------------------
# launch 1: logits + segment sums + mean
# ---------------------------------------------------------------------------

def _build_l1(pp):
    Kb = pp["Kb"]
    cb = pp["cb"]
    K = pp["K"]
    windows = pp["windows"]

    nc = bacc.Bacc("TRN2", target_bir_lowering=False, debug=False,
                   num_devices=NCORES)
    et_d = nc.dram_tensor("et", [P, K * H], dt.bfloat16,
                          kind="ExternalInput")
    xfT_d = nc.dram_tensor("xfT", [P, NPC], dt.float32,
                           kind="ExternalInput")
    recip_d = nc.dram_tensor("recip", [P, BPC], dt.float32,
                             kind="ExternalInput")
    wrepc_d = nc.dram_tensor("wrepc", [H, 1], dt.float32,
                             kind="ExternalInput")
    blead_d = nc.dram_tensor("blead", [1, 1], dt.float32,
                             kind="ExternalInput")

    logits_o = nc.dram_tensor("logits_o", [1, NPC], dt.float32,
                              kind="ExternalOutput")
    means_o = nc.dram_tensor("means_o", [P, BPC * H], dt.bfloat16,
                             kind="ExternalOutput")

    SB = 14                                        # blocks per means stage

    with tile.TileContext(nc) as tc:
        with (
            tc.tile_pool(name="const", bufs=1) as cp,
            tc.tile_pool(name="g", bufs=4) as gp,
            tc.tile_pool(name="stage", bufs=2) as stp,
            tc.tile_pool(name="sums_ps", bufs=4, space="PSUM") as sums_pp,
            tc.tile_pool(name="lg_ps", bufs=2, space="PSUM") as lg_pp,
        ):
            recip_t = cp.tile([P, BPC], dt.float32)
            nc.sync.dma_start(recip_t[:], recip_d[:, :])
            wrepc_t = cp.tile([H, 1], dt.float32)
            nc.sync.dma_start(wrepc_t[:], wrepc_d[:, :])
            blead_t = cp.tile([1, 1], dt.float32)
            nc.sync.dma_start(blead_t[:], blead_d[:, :])
            xfT_t = cp.tile([P, NPC], dt.float32)
            logits_sb = cp.tile([1, NPC], dt.float32)
            ident_f = cp.tile([P, P], dt.bfloat16)
            make_identity(nc, ident_f[:])

            stage_out = None
            for (bw0, bw1, col0, ncols) in windows:
                G = gp.tile([P, 56 * H], dt.bfloat16, tag="g")
                if ncols > 0:
                    nc.sync.dma_start(G[:, : ncols * H],
                                      et_d[:, col0 * H: (col0 + ncols) * H])
                q0w, q1w = bw0 * P, bw1 * P
                nc.sync.dma_start(xfT_t[:, q0w: q1w], xfT_d[:, q0w: q1w])
                for b in range(bw0, bw1):
                    sj = b % SB
                    if sj == 0:
                        stage_out = stp.tile([P, SB * H], dt.bfloat16,
                                             tag="st")
                    nb = int(Kb[b])
                    c0 = int(cb[b]) - col0
                    sums_ps = sums_pp.tile([P, H], dt.float32, space="PSUM",
                                           tag="sums")
                    if nb == 0:
                        nc.vector.memset(sums_ps[:], 0.0)
                    for j in range(nb):
                        nc.tensor.matmul(
                            out=sums_ps[:], lhsT=ident_f[:],
                            rhs=G[:, (c0 + j) * H: (c0 + j + 1) * H],
                            start=(j == 0), stop=(j == nb - 1))
                    nc.vector.tensor_scalar_mul(
                        stage_out[:, sj * H: (sj + 1) * H], sums_ps[:],
                        recip_t[:, b: b + 1])
                    if sj == SB - 1 or b == BPC - 1:
                        b0s = b - sj
                        nc.sync.dma_start(
                            means_o[:, b0s * H: (b + 1) * H],
                            stage_out[:, : (sj + 1) * H])
                # logits for this window's nodes: w_lead-stationary chunks
                for q0 in range(q0w, q1w, 448):
                    nq = min(448, q1w - q0)
                    lg_ps = lg_pp.tile([1, 448], dt.float32, space="PSUM",
                                       tag="lg")
                    nc.tensor.matmul(out=lg_ps[:, :nq], lhsT=wrepc_t[:],
                                     rhs=xfT_t[:, q0: q0 + nq],
                                     start=True, stop=True)
                    nc.scalar.activation(
                        logits_sb[:, q0: q0 + nq], lg_ps[:, :nq],
                        mybir.ActivationFunctionType.Identity,
                        bias=blead_t[:, :1])
            nc.sync.dma_start(logits_o[:, :], logits_sb[:])
    nc.compile()
    return nc


# ---------------------------------------------------------------------------
# launch 2: leader election + mean gather + MLP
# ---------------------------------------------------------------------------

def _view3(t, ngb, wc):
    """[P, ngb, wc] strided view of a [P, >=ngb*wc] tile."""
    a = t[:]
    return bass.AP(a.tensor, a.offset, [a.ap[0], [wc, ngb], [1, wc]])


def _build_l2(pp):
    chunks = pp["chunks"]
    SWR = pp["SWR"]
    WMAX = pp["WMAX"]
    mwin = pp["mwin"]
    MSCOLS = pp["MSCOLS"]

    nc = bacc.Bacc("TRN2", target_bir_lowering=False, debug=False,
                   num_devices=NCORES)
    SWR1 = max(SWR, 1)
    ep_d = nc.dram_tensor("epad", [P, SWR1], dt.float32,
                          kind="ExternalInput")
    sp1_d = nc.dram_tensor("srcp1", [P, SWR1], dt.float32,
                           kind="ExternalInput")
    means_d = nc.dram_tensor("meansfull", [NPAD, H], dt.bfloat16,
                             kind="ExternalInput")
    MSC1 = max(MSCOLS, 1)
    melog_d = nc.dram_tensor("melog", [P, MSC1], dt.float32,
                             kind="ExternalInput")
    mset_d = nc.dram_tensor("mset", [P, MSC1 * H], dt.bfloat16,
                            kind="ExternalInput")
    w1_d = nc.dram_tensor("w1", [H, H], dt.bfloat16, kind="ExternalInput")
    b1_d = nc.dram_tensor("b1", [P, 1], dt.float32, kind="ExternalInput")
    w2_d = nc.dram_tensor("w2", [H, C], dt.bfloat16, kind="ExternalInput")
    b2_d = nc.dram_tensor("b2c", [C, 1], dt.float32, kind="ExternalInput")
    out_o = nc.dram_tensor("out_o", [C, NPC], dt.bfloat16,
                           kind="ExternalOutput")

    MB = 4                                          # blocks per MLP group

    with tile.TileContext(nc) as tc:
        with (
            tc.tile_pool(name="const", bufs=1) as cp,
            tc.tile_pool(name="ein", bufs=2) as eip,
            tc.tile_pool(name="ework", bufs=2) as ewp,
            tc.tile_pool(name="lead", bufs=2) as ldp,
            tc.tile_pool(name="rows", bufs=2) as rp_,
            tc.tile_pool(name="win", bufs=max(len(mwin), 1)) as wnp,
            tc.tile_pool(name="gms", bufs=4) as gmp,
            tc.tile_pool(name="mw", bufs=3) as mwp,
            tc.tile_pool(name="mst", bufs=2) as mp,
            tc.tile_pool(name="ost", bufs=2) as stp,
            tc.tile_pool(name="ostm", bufs=2) as stp2,
            tc.tile_pool(name="tr_ps", bufs=2, space="PSUM") as tr_pp,
            tc.tile_pool(name="sel_ps", bufs=3, space="PSUM") as sel_pp,
            tc.tile_pool(name="mlp_ps", bufs=2, space="PSUM") as mlp_pp,
        ):
            w1_t = cp.tile([H, H], dt.bfloat16)
            nc.scalar.dma_start(w1_t[:], w1_d[:, :])
            b1_t = cp.tile([P, 1], dt.float32)
            nc.scalar.dma_start(b1_t[:], b1_d[:, :])
            w2_t = cp.tile([H, C], dt.bfloat16)
            nc.scalar.dma_start(w2_t[:], w2_d[:, :])
            b2_t = cp.tile([C, 1], dt.float32)
            nc.scalar.dma_start(b2_t[:], b2_d[:, :])
            ident_f = cp.tile([P, P], dt.bfloat16)
            make_identity(nc, ident_f[:])
            melog_t = cp.tile([P, MSC1], dt.float32)
            if MSCOLS > 0:
                nc.scalar.dma_start(melog_t[:], melog_d[:, :])

            def emit_mlp(meanT_stage, nmb, stage_out, g0):
                hpre_ps = mlp_pp.tile([P, MB * H], dt.float32,
                                      space="PSUM", tag="mlp")
                nc.tensor.matmul(out=hpre_ps[:, : nmb * H], lhsT=w1_t[:],
                                 rhs=meanT_stage[:, : nmb * P],
                                 start=True, stop=True)
                hT_stage = mp.tile([P, MB * H], dt.bfloat16, tag="hT")
                nc.scalar.activation(hT_stage[:, : nmb * H],
                                     hpre_ps[:, : nmb * H],
                                     mybir.ActivationFunctionType.Gelu,
                                     bias=b1_t[:, :1])
                rep_ps = mlp_pp.tile([P, MB * P], dt.float32,
                                     space="PSUM", tag="mlp")
                nc.tensor.matmul(out=rep_ps[:, : nmb * P], lhsT=w2_t[:],
                                 rhs=hT_stage[:, : nmb * H],
                                 start=True, stop=True)
                nc.scalar.activation(
                    stage_out[:, g0 * P: (g0 + nmb) * P],
                    rep_ps[:, : nmb * P],
                    mybir.ActivationFunctionType.Identity,
                    bias=b2_t[:, :1])

            # phase 1: indirect-side elections (DVE) -> per-chunk leadi
            leadis = []
            for (b0, ngb, Wc, coff) in chunks:
                n = ngb * Wc
                ep = eip.tile([P, CH * WMAX], dt.float32, tag="ep")
                nc.sync.dma_start(ep[:, :n], ep_d[:, coff: coff + n])
                epv = _view3(ep, ngb, Wc)
                sp1 = eip.tile([P, CH * WMAX], dt.float32, tag="sp1")
                nc.sync.dma_start(sp1[:, :n], sp1_d[:, coff: coff + n])
                sp1v = _view3(sp1, ngb, Wc)

                sm = ewp.tile([P, CH], dt.float32, tag="sm")
                nc.vector.reduce_max(out=sm[:, :ngb], in_=epv,
                                     axis=mybir.AxisListType.X)
                mask = ewp.tile([P, CH * WMAX], dt.float32, tag="mask")
                maskv = _view3(mask, ngb, Wc)
                a = sm[:]
                sm_b = bass.AP(a.tensor, a.offset,
                               [a.ap[0], [1, ngb], [0, Wc]])
                nc.vector.tensor_tensor(out=maskv, in0=epv, in1=sm_b,
                                        op=mybir.AluOpType.is_equal)
                cand = ewp.tile([P, CH * WMAX], dt.float32, tag="cand")
                candv = _view3(cand, ngb, Wc)
                nc.vector.tensor_tensor(out=candv, in0=maskv, in1=sp1v,
                                        op=mybir.AluOpType.mult)
                lp1 = ewp.tile([P, CH], dt.float32, tag="lp1")
                nc.vector.reduce_max(out=lp1[:, :ngb], in_=candv,
                                     axis=mybir.AxisListType.X)
                leadf = ewp.tile([P, CH], dt.float32, tag="leadf")
                nc.vector.tensor_scalar(
                    out=leadf[:, :ngb], in0=lp1[:, :ngb], scalar1=-1.0,
                    scalar2=0.0,
                    op0=mybir.AluOpType.add, op1=mybir.AluOpType.max)
                leadi = ldp.tile([P, CH], dt.int32, tag="leadi")
                nc.vector.tensor_copy(leadi[:, :ngb], leadf[:, :ngb])
                leadis.append(leadi)

            # phase 2: issue all indirect gathers (GpSimd self-paces; they
            # drain after the mset stream quiesces)
            rows_l = []
            for ci, (b0, ngb, Wc, coff) in enumerate(chunks):
                leadi = leadis[ci]
                rows = rp_.tile([P, CH, H], dt.bfloat16, tag="rows")
                for j in range(ngb):
                    nc.gpsimd.indirect_dma_start(
                        out=rows[:, j, :],
                        out_offset=None,
                        in_=means_d[:, :],
                        in_offset=bass.IndirectOffsetOnAxis(
                            ap=leadi[:, j: j + 1], axis=0),
                    )
                rows_l.append(rows)

            # phase 3: masked-side winner masks (DVE)
            wins = []
            for (b0m, nblk, Kw, col0m) in mwin:
                nmc = nblk * Kw
                smx = ewp.tile([P, MSW], dt.float32, tag="smx")
                ml = melog_t[:, col0m: col0m + nmc]
                lv = bass.AP(ml.tensor, ml.offset,
                             [ml.ap[0], [Kw, nblk], [1, Kw]])
                nc.vector.reduce_max(out=smx[:, :nblk], in_=lv,
                                     axis=mybir.AxisListType.X)
                win = wnp.tile([P, MSW], dt.bfloat16, tag="win")
                winv = _view3(win, nblk, Kw)
                a = smx[:]
                smx_b = bass.AP(a.tensor, a.offset,
                                [a.ap[0], [1, nblk], [0, Kw]])
                nc.vector.tensor_tensor(out=winv, in0=lv, in1=smx_b,
                                        op=mybir.AluOpType.is_equal)
                wins.append(win)

            # phase 4: mset stream + select + MLP
            gp_elems, dve_elems = 0, 1
            for wi, (b0m, nblk, Kw, col0m) in enumerate(mwin):
                nmc = nblk * Kw
                G = gmp.tile([P, MSW * H], dt.bfloat16, tag="gms")
                nc.sync.dma_start(G[:, : nmc * H],
                                  mset_d[:, col0m * H: (col0m + nmc) * H])
                win = wins[wi]
                mw = mwp.tile([P, MSW * H], dt.bfloat16, tag="mw")
                ga = G[:, : nmc * H]
                gv = bass.AP(ga.tensor, ga.offset,
                             [ga.ap[0], [H, nmc], [1, H]])
                wa = win[:, : nmc]
                wv = bass.AP(wa.tensor, wa.offset,
                             [wa.ap[0], [1, nmc], [0, H]])
                ma = mw[:, : nmc * H]
                mv = bass.AP(ma.tensor, ma.offset,
                             [ma.ap[0], [H, nmc], [1, H]])
                if gp_elems * 2 < dve_elems:
                    eng = nc.gpsimd
                    gp_elems += nmc
                else:
                    eng = nc.vector
                    dve_elems += nmc
                eng.tensor_tensor(out=mv, in0=gv, in1=wv,
                                  op=mybir.AluOpType.mult)

                stage_out = stp2.tile([P, 12 * P], dt.bfloat16, tag="stm")
                meanT_stage = None
                for bi in range(nblk):
                    gj = bi % MB
                    if gj == 0:
                        meanT_stage = mp.tile([P, MB * P], dt.bfloat16,
                                              tag="meanT")
                    sel_ps = sel_pp.tile([P, 512], dt.float32, space="PSUM",
                                         tag="sel")
                    for j in range(Kw):
                        c = bi * Kw + j
                        nc.tensor.matmul(out=sel_ps[:, :P],
                                         lhsT=mw[:, c * H: (c + 1) * H],
                                         rhs=ident_f[:],
                                         start=(j == 0), stop=(j == Kw - 1))
                    nc.scalar.activation(
                        meanT_stage[:, gj * P: (gj + 1) * P], sel_ps[:, :P],
                        mybir.ActivationFunctionType.Copy)
                    if gj == MB - 1 or bi == nblk - 1:
                        emit_mlp(meanT_stage, gj + 1, stage_out, bi - gj)
                nc.scalar.dma_start(out_o[:, b0m * P: (b0m + nblk) * P],
                                    stage_out[:, : nblk * P])

            # phase 5: indirect-side MLP
            for ci, (b0, ngb, Wc, coff) in enumerate(chunks):
                rows = rows_l[ci]
                stage_out = stp.tile([P, CH * P], dt.bfloat16, tag="st")
                for g0 in range(0, ngb, MB):
                    nmb = min(MB, ngb - g0)
                    meanT_stage = mp.tile([P, MB * P], dt.bfloat16,
                                          tag="meanT")
                    for j in range(nmb):
                        meanT_ps = tr_pp.tile([P, P], dt.bfloat16,
                                              space="PSUM", tag="tr")
                        nc.tensor.transpose(meanT_ps[:], rows[:, g0 + j, :],
                                            ident_f[:])
                        nc.scalar.activation(
                            meanT_stage[:, j * P: (j + 1) * P], meanT_ps[:],
                            mybir.ActivationFunctionType.Copy)
                    emit_mlp(meanT_stage, nmb, stage_out, g0)
                nc.scalar.dma_start(out_o[:, b0 * P: (b0 + ngb) * P],
                                    stage_out[:, : ngb * P])
    nc.compile()
    return nc


# ---------------------------------------------------------------------------

_CACHE = {}


def _get(key, fn):
    if key not in _CACHE:
        _CACHE[key] = fn()
    return _CACHE[key]


def kernel(x, edge_index, w_lead, b_lead, w1, b1, w2, b2):
    x = np.asarray(x, np.float32)
    assert x.shape == (N_NODES, H)

    ekey = hashlib.md5(np.asarray(edge_index).tobytes()).hexdigest()
    pp = _get(("pp", ekey), lambda: _preprocess(edge_index))

    K = pp["K"]
    inv = pp["inv"]
    inv_flat = inv.reshape(NCORES, NPC)

    xbf_ext = np.zeros((N_NODES + 1, H), bf16)
    xbf_ext[:N_NODES] = x.astype(bf16)
    xpad = np.zeros((NPAD, H), np.float32)
    xpad[:N_NODES] = x

    wrepc = np.ascontiguousarray(
        np.asarray(w_lead, np.float32).reshape(H, 1))
    blead = np.full((1, 1), np.float32(b_lead), np.float32)
    w1f = np.ascontiguousarray(np.asarray(w1, np.float32).astype(bf16))
    b1c = np.ascontiguousarray(np.asarray(b1, np.float32).reshape(H, 1))
    w2f = np.ascontiguousarray(np.asarray(w2, np.float32).astype(bf16))
    b2c = np.ascontiguousarray(np.asarray(b2, np.float32).reshape(C, 1))

    # ---- launch 1 ----------------------------------------------------------
    nc1 = _get(("l1", ekey), lambda: _build_l1(pp))
    in_maps = []
    for k in range(NCORES):
        # ET[k]: [P, K*H] partition-major slot table (slot = c*128+p)
        et = xbf_ext[pp["srcidx"][k]].reshape(K, P, H).transpose(1, 0, 2)
        et = np.ascontiguousarray(et).reshape(P, K * H)
        # xfT: [H, NPC] owned-node features transposed (f32, for logits)
        xf = np.where((inv[k] >= 0)[:, :, None],
                      xpad[np.maximum(inv[k], 0)], 0.0)  # [BPC, P, H]
        xfT = np.ascontiguousarray(
            xf.reshape(NPC, H).T.astype(np.float32))
        in_maps.append({
            "et": et,
            "xfT": xfT,
            "recip": pp["recip"][k],
            "wrepc": wrepc,
            "blead": blead,
        })
    r1 = run_bass_kernel_spmd(nc1, in_maps, core_ids=CORES)

    logits_full = np.zeros(NPAD, np.float32)
    means_full = np.zeros((NPAD, H), bf16)
    for k in range(NCORES):
        lg = r1.results[k]["logits_o"].reshape(NPC)     # [b*128+p]
        mn = r1.results[k]["means_o"].reshape(P, BPC, H)
        mf = inv_flat[k] >= 0
        ids = inv_flat[k][mf]
        logits_full[ids] = lg[mf]
        m = inv[k] >= 0                                 # [BPC, P]
        means_full[inv[k][m]] = mn.transpose(1, 0, 2)[m]

    # ---- launch 2: election + mean gather + MLP ----------------------------
    nc2 = _get(("l2", ekey), lambda: _build_l2(pp))
    es = pp["elog_src"]
    msrc = pp["msrc"]
    MSCOLS = pp["MSCOLS"]
    logits_ext = np.full(NPAD + 1, NEG, np.float32)
    logits_ext[:NPAD] = logits_full
    in_maps2 = []
    zpad = np.zeros((P, 1), np.float32)
    for k in range(NCORES):
        if pp["SWR"] > 0:
            ep = np.ascontiguousarray(
                np.where(es[k] >= 0, logits_full[np.maximum(es[k], 0)],
                         NEG).astype(np.float32))
            sp1k = pp["srcp1"][k]
        else:
            ep, sp1k = zpad, zpad
        if MSCOLS > 0:
            melog = np.where(msrc[k] < N_NODES, logits_ext[msrc[k]],
                             NEG).astype(np.float32)
            mset = means_full[msrc[k]].reshape(P, MSCOLS * H)
        else:
            melog = zpad
            mset = np.zeros((P, H), bf16)
        in_maps2.append({
            "epad": ep,
            "srcp1": sp1k,
            "meansfull": means_full,
            "melog": np.ascontiguousarray(melog),
            "mset": np.ascontiguousarray(mset),
            "w1": w1f,
            "b1": b1c,
            "w2": w2f,
            "b2c": b2c,
        })
    r2 = run_bass_kernel_spmd(nc2, in_maps2, core_ids=CORES)

    out = np.zeros((N_NODES, C), np.float32)
    for k in range(NCORES):
        g = r2.results[k]["out_o"].reshape(C, BPC, P).astype(np.float32)
        node_rows = g.transpose(1, 2, 0).reshape(NPC, C)
        m = inv_flat[k] >= 0
        out[inv_flat[k][m]] = node_rows[m]
    return out


# revision 37
# speedup vs baseline: 1.1557x; 1.0029x over previous
"""Trainium2 Bass kernel for nn_DecentralizedCoordinator (GNN message passing).

Strategy (8 NeuronCores, SPMD), v11:
- Nodes degree-sorted and round-robin sharded: global in-degree rank r ->
  core r%8, block (r//8)//128, slot (r//8)%128. Each block then holds
  same-degree nodes, so the edge-source feature table ET needs exactly
  Kb = max-indeg-in-block identity-lhsT matmul columns per block: no
  one-hot tail region at all, ~1.7% padding, identical layout per core.
- Launch 1 (DMA-bound, ~120us): leader logits via w_lead-stationary
  matmuls over an f32 xfT stream interleaved with the ET windows;
  segment sums via identity-lhsT matmul accumulation over the streamed
  bf16 ET; mean = sums * recip(indeg) on DVE -> means (bf16, dst-major)
  + logits (f32) to DRAM.
- Host between launches (index-pattern reshuffles only): assemble the
  means_full[node] table and the ragged per-chunk padded election layout
  epad/srcp1 from logits.
- Launch 2 (GpSimd descriptor-gen bound, ~162us): leader election with
  the exact reference tie-break (seg-max logits, then max src id) on
  DVE; back-to-back indirect-DMA gathers of each dst's LEADER'S MEAN row
  (98 x 128-row gathers, the Q7 descriptor-generation floor); then
  transpose + MLP (w1 -> gelu+b1 -> w2 -> +b2) on the gathered rows,
  with PE/Scalar work hidden under the gathers.
  (NB_IND < 98 enables an experimental masked-sum path for low-degree
  blocks; it is disabled -- concurrent streams starve the Q7 descriptor
  drain and showed rare nondeterministic corruption on cold runs.)

Host only shards/permutes/reshuffles by precomputed index patterns; every
operation on runtime values (logits, sums, means, MLP, comparisons,
election, the leader gather) is on device.
"""
import hashlib
import sys

import numpy as np
import ml_dtypes

sys.path.insert(0, "/opt/trn_rl_repo")

import concourse.bass as bass
import concourse.tile as tile
from concourse import bacc, mybir
from concourse.bass_utils import run_bass_kernel_spmd
from concourse.masks import make_identity

dt = mybir.dt
bf16 = ml_dtypes.bfloat16

P = 128
NCORES = 8
BPC = 98                 # dst blocks per core
NPC = BPC * P            # 12544 nodes per core
NPAD = NCORES * NPC      # 100352 padded node count
N_NODES = 100000
H = 128
C = 128
NEG = -3.0e38
CH = 16                  # blocks per launch-2 indirect chunk
NB_IND = 98              # blocks on the indirect-gather path (rest: masked)
MSW = 72                 # masked-window column budget

CORES = list(range(NCORES))


def _preprocess(edge_index):
    row = np.asarray(edge_index[0], np.int64)
    col = np.asarray(edge_index[1], np.int64)
    E = len(row)

    indeg = np.bincount(col, minlength=N_NODES)

    # degree-sorted round-robin assignment: rank r -> (r%8, (r//8)//128,
    # (r//8)%128)
    order = np.argsort(-indeg, kind="stable")          # rank -> node
    rr = np.arange(N_NODES)
    kk_of_rank = rr % NCORES
    pos = rr // NCORES
    bb_of_rank = pos // P
    pp_of_rank = pos % P
    node2kbp = np.zeros((N_NODES, 3), np.int64)
    node2kbp[order, 0] = kk_of_rank
    node2kbp[order, 1] = bb_of_rank
    node2kbp[order, 2] = pp_of_rank
    inv = np.full((NCORES, BPC, P), -1, np.int64)
    inv[kk_of_rank, bb_of_rank, pp_of_rank] = order

    # per-block column count (shared across cores): max indeg in the
    # 1024-rank group = indeg of its first (sorted desc)
    Kb = np.zeros(BPC, np.int64)
    for b in range(BPC):
        r0 = b * NCORES * P
        Kb[b] = indeg[order[r0]] if r0 < N_NODES else 0
    cb = np.concatenate([[0], np.cumsum(Kb)])          # col offset per block
    K = int(cb[-1])

    # edges grouped by dst, rank within dst
    dorder = np.argsort(col, kind="stable")
    row_d = row[dorder]
    col_d = col[dorder]
    dst_starts = np.concatenate([[0], np.cumsum(indeg)])
    ranks = np.arange(E) - dst_starts[col_d]

    kk = node2kbp[col_d, 0]
    bb = node2kbp[col_d, 1]
    ppos = node2kbp[col_d, 2]

    srcidx = np.full((NCORES, K * P), N_NODES, np.int64)
    srcidx[kk, (cb[bb] + ranks) * P + ppos] = row_d

    # recip of true in-degree per owned node, [NCORES, P, BPC]
    cnt = np.where(inv >= 0, indeg[np.maximum(inv, 0)], 0.0)   # [NC,BPC,P]
    recip = np.ascontiguousarray(
        (1.0 / np.maximum(cnt, 1.0)).transpose(0, 2, 1)).astype(np.float32)

    # ET stream windows: whole blocks, ~56 cols each
    windows = []           # (b_start, b_end, col0, ncols)
    b = 0
    while b < BPC:
        b1 = b
        ncols = 0
        while b1 < BPC and (ncols + Kb[b1] <= 56 or b1 == b):
            ncols += int(Kb[b1])
            b1 += 1
        windows.append((b, b1, int(cb[b]), ncols))
        b = b1

    # ragged election layout (indirect blocks only): chunk c covers CH
    # blocks, width Wc = max ext-degree (indeg+1) in chunk
    chunks = []            # (b0, ngb, Wc, off)
    off = 0
    for b0 in range(0, NB_IND, CH):
        ngb = min(CH, NB_IND - b0)
        r0 = b0 * NCORES * P
        Wc = int(indeg[order[r0]]) + 1 if r0 < N_NODES else 1
        chunks.append((b0, ngb, Wc, off))
        off += ngb * Wc
    SWR = off

    WMAX = max([w for (_, _, w, _) in chunks], default=1)
    elog_src = np.full((NCORES, P, SWR), -1, np.int64)
    for (b0, ngb, Wc, coff) in chunks:
        for bi in range(ngb):
            b = b0 + bi
            base = coff + bi * Wc
            for k in range(NCORES):
                for p in range(P):
                    d = int(inv[k, b, p])
                    if d < 0:
                        continue
                    s0, s1 = int(dst_starts[d]), int(dst_starts[d + 1])
                    m = s1 - s0
                    elog_src[k, p, base] = d
                    if m > 0:
                        elog_src[k, p, base + 1: base + 1 + m] = row_d[s0:s1]
    srcp1 = np.where(elog_src >= 0, elog_src + 1, 0).astype(np.float32)

    # ---- masked-sum side (blocks NB_IND..BPC) -------------------------
    # per-dst dedup candidate list: [self] + unique in-edge srcs != self
    srt = np.lexsort((row_d, col_d))
    c2, r2 = col_d[srt], row_d[srt]
    uniq = np.ones(E, bool)
    uniq[1:] = (c2[1:] != c2[:-1]) | (r2[1:] != r2[:-1])
    uniq &= (r2 != c2)
    cu, ru = c2[uniq], r2[uniq]
    dl = np.bincount(cu, minlength=N_NODES)            # dedup in-deg (no self)
    du_starts = np.concatenate([[0], np.cumsum(dl)])

    # per-block candidate width (shared across cores)
    wd = np.zeros(BPC, np.int64)
    for b in range(NB_IND, BPC):
        r0, r1 = b * NCORES * P, min((b + 1) * NCORES * P, N_NODES)
        wd[b] = (dl[order[r0:r1]].max() + 1) if r1 > r0 else 1

    # masked windows: blocks with uniform padded width Kw, nblk*Kw <= MSW
    mwin = []              # (b0, nblk, Kw, col0)
    b = NB_IND
    mc = 0
    while b < BPC:
        b1 = b
        Kw = int(wd[b])
        while b1 < BPC:
            nKw = max(Kw, int(wd[b1]))
            if ((b1 - b + 1) * nKw > MSW or b1 - b >= 12) and b1 > b:
                break
            Kw = nKw
            b1 += 1
        mwin.append((b, b1 - b, Kw, mc))
        mc += (b1 - b) * Kw
        b = b1
    MSCOLS = mc

    msrc = np.full((NCORES, P, MSCOLS), N_NODES, np.int64)
    for (b0m, nblk, Kw, col0m) in mwin:
        for bi in range(nblk):
            b = b0m + bi
            base = col0m + bi * Kw
            for k in range(NCORES):
                for p in range(P):
                    d = int(inv[k, b, p])
                    if d < 0:
                        continue
                    msrc[k, p, base] = d
                    s0, s1 = int(du_starts[d]), int(du_starts[d + 1])
                    m = s1 - s0
                    if m > 0:
                        msrc[k, p, base + 1: base + 1 + m] = ru[s0:s1]

    return dict(
        Kb=Kb, cb=cb, K=K, windows=windows,
        srcidx=srcidx, recip=recip,
        chunks=chunks, SWR=SWR, WMAX=WMAX, elog_src=elog_src, srcp1=srcp1,
        mwin=mwin, MSCOLS=MSCOLS, msrc=msrc,
        node2kbp=node2kbp, inv=inv,
    )


# ---------------------------------------------------------------------------
# launch 1: logits + segment sums + mean
# ---------------------------------------------------------------------------

def _build_l1(pp):
    Kb = pp["Kb"]
    cb = pp["cb"]
    K = pp["K"]
    windows = pp["windows"]

    nc = bacc.Bacc("TRN2", target_bir_lowering=False, debug=False,
                   num_devices=NCORES)
    et_d = nc.dram_tensor("et", [P, K * H], dt.bfloat16,
                          kind="ExternalInput")
    xfT_d = nc.dram_tensor("xfT", [P, NPC], dt.float32,
                           kind="ExternalInput")
    recip_d = nc.dram_tensor("recip", [P, BPC], dt.float32,
                             kind="ExternalInput")
    wrepc_d = nc.dram_tensor("wrepc", [H, 1], dt.float32,
                             kind="ExternalInput")
    blead_d = nc.dram_tensor("blead", [1, 1], dt.float32,
                             kind="ExternalInput")

    logits_o = nc.dram_tensor("logits_o", [1, NPC], dt.float32,
                              kind="ExternalOutput")
    means_o = nc.dram_tensor("means_o", [P, BPC * H], dt.bfloat16,
                             kind="ExternalOutput")

    SB = 14                                        # blocks per means stage

    with tile.TileContext(nc) as tc:
        with (
            tc.tile_pool(name="const", bufs=1) as cp,
            tc.tile_pool(name="g", bufs=4) as gp,
            tc.tile_pool(name="stage", bufs=2) as stp,
            tc.tile_pool(name="sums_ps", bufs=4, space="PSUM") as sums_pp,
            tc.tile_pool(name="lg_ps", bufs=2, space="PSUM") as lg_pp,
        ):
            recip_t = cp.tile([P, BPC], dt.float32)
            nc.sync.dma_start(recip_t[:], recip_d[:, :])
            wrepc_t = cp.tile([H, 1], dt.float32)
            nc.sync.dma_start(wrepc_t[:], wrepc_d[:, :])
            blead_t = cp.tile([1, 1], dt.float32)
            nc.sync.dma_start(blead_t[:], blead_d[:, :])
            xfT_t = cp.tile([P, NPC], dt.float32)
            logits_sb = cp.tile([1, NPC], dt.float32)
            ident_f = cp.tile([P, P], dt.bfloat16)
            make_identity(nc, ident_f[:])

            stage_out = None
            for (bw0, bw1, col0, ncols) in windows:
                G = gp.tile([P, 56 * H], dt.bfloat16, tag="g")
                if ncols > 0:
                    nc.sync.dma_start(G[:, : ncols * H],
                                      et_d[:, col0 * H: (col0 + ncols) * H])
                q0w, q1w = bw0 * P, bw1 * P
                nc.sync.dma_start(xfT_t[:, q0w: q1w], xfT_d[:, q0w: q1w])
                for b in range(bw0, bw1):
                    sj = b % SB
                    if sj == 0:
                        stage_out = stp.tile([P, SB * H], dt.bfloat16,
                                             tag="st")
                    nb = int(Kb[b])
                    c0 = int(cb[b]) - col0
                    sums_ps = sums_pp.tile([P, H], dt.float32, space="PSUM",
                                           tag="sums")
                    if nb == 0:
                        nc.vector.memset(sums_ps[:], 0.0)
                    for j in range(nb):
                        nc.tensor.matmul(
                            out=sums_ps[:], lhsT=ident_f[:],
                            rhs=G[:, (c0 + j) * H: (c0 + j + 1) * H],
                            start=(j == 0), stop=(j == nb - 1))
                    nc.vector.tensor_scalar_mul(
                        stage_out[:, sj * H: (sj + 1) * H], sums_ps[:],
                        recip_t[:, b: b + 1])
                    if sj == SB - 1 or b == BPC - 1:
                        b0s = b - sj
                        nc.sync.dma_start(
                            means_o[:, b0s * H: (b + 1) * H],
                            stage_out[:, : (sj + 1) * H])
                # logits for this window's nodes: w_lead-stationary chunks
                for q0 in range(q0w, q1w, 448):
                    nq = min(448, q1w - q0)
                    lg_ps = lg_pp.tile([1, 448], dt.float32, space="PSUM",
                                       tag="lg")
                    nc.tensor.matmul(out=lg_ps[:, :nq], lhsT=wrepc_t[:],
                                     rhs=xfT_t[:, q0: q0 + nq],
                                     start=True, stop=True)
                    nc.scalar.activation(
                        logits_sb[:, q0: q0 + nq], lg_ps[:, :nq],
                        mybir.ActivationFunctionType.Identity,
                        bias=blead_t[:, :1])
            nc.sync.dma_start(logits_o[:, :], logits_sb[:])
    nc.compile()
    return nc


# ---------------------------------------------------------------------------
# launch 2: leader election + mean gather + MLP
# ---------------------------------------------------------------------------

def _view3(t, ngb, wc):
    """[P, ngb, wc] strided view of a [P, >=ngb*wc] tile."""
    a = t[:]
    return bass.AP(a.tensor, a.offset, [a.ap[0], [wc, ngb], [1, wc]])


def _build_l2(pp):
    chunks = pp["chunks"]
    SWR = pp["SWR"]
    WMAX = pp["WMAX"]
    mwin = pp["mwin"]
    MSCOLS = pp["MSCOLS"]

    nc = bacc.Bacc("TRN2", target_bir_lowering=False, debug=False,
                   num_devices=NCORES)
    SWR1 = max(SWR, 1)
    ep_d = nc.dram_tensor("epad", [P, SWR1], dt.float32,
                          kind="ExternalInput")
    sp1_d = nc.dram_tensor("srcp1", [P, SWR1], dt.float32,
                           kind="ExternalInput")
    means_d = nc.dram_tensor("meansfull", [NPAD, H], dt.bfloat16,
                             kind="ExternalInput")
    MSC1 = max(MSCOLS, 1)
    melog_d = nc.dram_tensor("melog", [P, MSC1], dt.float32,
                             kind="ExternalInput")
    mset_d = nc.dram_tensor("mset", [P, MSC1 * H], dt.bfloat16,
                            kind="ExternalInput")
    w1_d = nc.dram_tensor("w1", [H, H], dt.bfloat16, kind="ExternalInput")
    b1_d = nc.dram_tensor("b1", [P, 1], dt.float32, kind="ExternalInput")
    w2_d = nc.dram_tensor("w2", [H, C], dt.bfloat16, kind="ExternalInput")
    b2_d = nc.dram_tensor("b2c", [C, 1], dt.float32, kind="ExternalInput")
    out_o = nc.dram_tensor("out_o", [C, NPC], dt.bfloat16,
                           kind="ExternalOutput")

    MB = 4                                          # blocks per MLP group

    with tile.TileContext(nc) as tc:
        with (
            tc.tile_pool(name="const", bufs=1) as cp,
            tc.tile_pool(name="ein", bufs=2) as eip,
            tc.tile_pool(name="ework", bufs=2) as ewp,
            tc.tile_pool(name="lead", bufs=2) as ldp,
            tc.tile_pool(name="rows", bufs=2) as rp_,
            tc.tile_pool(name="win", bufs=max(len(mwin), 1)) as wnp,
            tc.tile_pool(name="gms", bufs=4) as gmp,
            tc.tile_pool(name="mw", bufs=3) as mwp,
            tc.tile_pool(name="mst", bufs=2) as mp,
            tc.tile_pool(name="ost", bufs=2) as stp,
            tc.tile_pool(name="ostm", bufs=2) as stp2,
            tc.tile_pool(name="tr_ps", bufs=2, space="PSUM") as tr_pp,
            tc.tile_pool(name="sel_ps", bufs=3, space="PSUM") as sel_pp,
            tc.tile_pool(name="mlp_ps", bufs=2, space="PSUM") as mlp_pp,
        ):
            w1_t = cp.tile([H, H], dt.bfloat16)
            nc.scalar.dma_start(w1_t[:], w1_d[:, :])
            b1_t = cp.tile([P, 1], dt.float32)
            nc.scalar.dma_start(b1_t[:], b1_d[:, :])
            w2_t = cp.tile([H, C], dt.bfloat16)
            nc.scalar.dma_start(w2_t[:], w2_d[:, :])
            b2_t = cp.tile([C, 1], dt.float32)
            nc.scalar.dma_start(b2_t[:], b2_d[:, :])
            ident_f = cp.tile([P, P], dt.bfloat16)
            make_identity(nc, ident_f[:])
            melog_t = cp.tile([P, MSC1], dt.float32)
            if MSCOLS > 0:
                nc.scalar.dma_start(melog_t[:], melog_d[:, :])

            def emit_mlp(meanT_stage, nmb, stage_out, g0):
                hpre_ps = mlp_pp.tile([P, MB * H], dt.float32,
                                      space="PSUM", tag="mlp")
                nc.tensor.matmul(out=hpre_ps[:, : nmb * H], lhsT=w1_t[:],
                                 rhs=meanT_stage[:, : nmb * P],
                                 start=True, stop=True)
                hT_stage = mp.tile([P, MB * H], dt.bfloat16, tag="hT")
                nc.scalar.activation(hT_stage[:, : nmb * H],
                                     hpre_ps[:, : nmb * H],
                                     mybir.ActivationFunctionType.Gelu,
                                     bias=b1_t[:, :1])
                rep_ps = mlp_pp.tile([P, MB * P], dt.float32,
                                     space="PSUM", tag="mlp")
                nc.tensor.matmul(out=rep_ps[:, : nmb * P], lhsT=w2_t[:],
                                 rhs=hT_stage[:, : nmb * H],
                                 start=True, stop=True)
                nc.scalar.activation(
                    stage_out[:, g0 * P: (g0 + nmb) * P],
                    rep_ps[:, : nmb * P],
                    mybir.ActivationFunctionType.Identity,
                    bias=b2_t[:, :1])

            # phase 1: indirect-side elections (DVE) -> per-chunk leadi
            leadis = []
            for (b0, ngb, Wc, coff) in chunks:
                n = ngb * Wc
                ep = eip.tile([P, CH * WMAX], dt.float32, tag="ep")
                nc.sync.dma_start(ep[:, :n], ep_d[:, coff: coff + n])
                epv = _view3(ep, ngb, Wc)
                sp1 = eip.tile([P, CH * WMAX], dt.float32, tag="sp1")
                nc.sync.dma_start(sp1[:, :n], sp1_d[:, coff: coff + n])
                sp1v = _view3(sp1, ngb, Wc)

                sm = ewp.tile([P, CH], dt.float32, tag="sm")
                nc.vector.reduce_max(out=sm[:, :ngb], in_=epv,
                                     axis=mybir.AxisListType.X)
                mask = ewp.tile([P, CH * WMAX], dt.float32, tag="mask")
                maskv = _view3(mask, ngb, Wc)
                a = sm[:]
                sm_b = bass.AP(a.tensor, a.offset,
                               [a.ap[0], [1, ngb], [0, Wc]])
                nc.vector.tensor_tensor(out=maskv, in0=epv, in1=sm_b,
                                        op=mybir.AluOpType.is_equal)
                cand = ewp.tile([P, CH * WMAX], dt.float32, tag="cand")
                candv = _view3(cand, ngb, Wc)
                nc.vector.tensor_tensor(out=candv, in0=maskv, in1=sp1v,
                                        op=mybir.AluOpType.mult)
                lp1 = ewp.tile([P, CH], dt.float32, tag="lp1")
                nc.vector.reduce_max(out=lp1[:, :ngb], in_=candv,
                                     axis=mybir.AxisListType.X)
                leadf = ewp.tile([P, CH], dt.float32, tag="leadf")
                nc.vector.tensor_scalar(
                    out=leadf[:, :ngb], in0=lp1[:, :ngb], scalar1=-1.0,
                    scalar2=0.0,
                    op0=mybir.AluOpType.add, op1=mybir.AluOpType.max)
                leadi = ldp.tile([P, CH], dt.int32, tag="leadi")
                nc.vector.tensor_copy(leadi[:, :ngb], leadf[:, :ngb])
                leadis.append(leadi)

            # phase 2: issue all indirect gathers (GpSimd self-paces; they
            # drain after the mset stream quiesces)
            rows_l = []
            for ci, (b0, ngb, Wc, coff) in enumerate(chunks):
                leadi = leadis[ci]
                rows = rp_.tile([P, CH, H], dt.bfloat16, tag="rows")
                for j in range(ngb):
                    nc.gpsimd.indirect_dma_start(
                        out=rows[:, j, :],
                        out_offset=None,
                        in_=means_d[:, :],
                        in_offset=bass.IndirectOffsetOnAxis(
                            ap=leadi[:, j: j + 1], axis=0),
                    )
                rows_l.append(rows)

            # phase 3: masked-side winner masks (DVE)
            wins = []
            for (b0m, nblk, Kw, col0m) in mwin:
                nmc = nblk * Kw
                smx = ewp.tile([P, MSW], dt.float32, tag="smx")
                ml = melog_t[:, col0m: col0m + nmc]
                lv = bass.AP(ml.tensor, ml.offset,
                             [ml.ap[0], [Kw, nblk], [1, Kw]])
                nc.vector.reduce_max(out=smx[:, :nblk], in_=lv,
                                     axis=mybir.AxisListType.X)
                win = wnp.tile([P, MSW], dt.bfloat16, tag="win")
                winv = _view3(win, nblk, Kw)
                a = smx[:]
                smx_b = bass.AP(a.tensor, a.offset,
                                [a.ap[0], [1, nblk], [0, Kw]])
                nc.vector.tensor_tensor(out=winv, in0=lv, in1=smx_b,
                                        op=mybir.AluOpType.is_equal)
                wins.append(win)

            # phase 4: mset stream + select + MLP
            gp_elems, dve_elems = 0, 1
            for wi, (b0m, nblk, Kw, col0m) in enumerate(mwin):
                nmc = nblk * Kw
                G = gmp.tile([P, MSW * H], dt.bfloat16, tag="gms")
                nc.sync.dma_start(G[:, : nmc * H],
                                  mset_d[:, col0m * H: (col0m + nmc) * H])
                win = wins[wi]
                mw = mwp.tile([P, MSW * H], dt.bfloat16, tag="mw")
                ga = G[:, : nmc * H]
                gv = bass.AP(ga.tensor, ga.offset,
                             [ga.ap[0], [H, nmc], [1, H]])
                wa = win[:, : nmc]
                wv = bass.AP(wa.tensor, wa.offset,
                             [wa.ap[0], [1, nmc], [0, H]])
                ma = mw[:, : nmc * H]
                mv = bass.AP(ma.tensor, ma.offset,
                             [ma.ap[0], [H, nmc], [1, H]])
                if gp_elems * 2 < dve_elems:
                    eng = nc.gpsimd
                    gp_elems += nmc
                else:
                    eng = nc.vector
                    dve_elems += nmc
                eng.tensor_tensor(out=mv, in0=gv, in1=wv,
                                  op=mybir.AluOpType.mult)

                stage_out = stp2.tile([P, 12 * P], dt.bfloat16, tag="stm")
                meanT_stage = None
                for bi in range(nblk):
                    gj = bi % MB
                    if gj == 0:
                        meanT_stage = mp.tile([P, MB * P], dt.bfloat16,
                                              tag="meanT")
                    sel_ps = sel_pp.tile([P, 512], dt.float32, space="PSUM",
                                         tag="sel")
                    for j in range(Kw):
                        c = bi * Kw + j
                        nc.tensor.matmul(out=sel_ps[:, :P],
                                         lhsT=mw[:, c * H: (c + 1) * H],
                                         rhs=ident_f[:],
                                         start=(j == 0), stop=(j == Kw - 1))
                    nc.scalar.activation(
                        meanT_stage[:, gj * P: (gj + 1) * P], sel_ps[:, :P],
                        mybir.ActivationFunctionType.Copy)
                    if gj == MB - 1 or bi == nblk - 1:
                        emit_mlp(meanT_stage, gj + 1, stage_out, bi - gj)
                nc.scalar.dma_start(out_o[:, b0m * P: (b0m + nblk) * P],
                                    stage_out[:, : nblk * P])

            # phase 5: indirect-side MLP
            for ci, (b0, ngb, Wc, coff) in enumerate(chunks):
                rows = rows_l[ci]
                stage_out = stp.tile([P, CH * P], dt.bfloat16, tag="st")
                for g0 in range(0, ngb, MB):
                    nmb = min(MB, ngb - g0)
                    meanT_stage = mp.tile([P, MB * P], dt.bfloat16,
                                          tag="meanT")
                    for j in range(nmb):
                        meanT_ps = tr_pp.tile([P, P], dt.bfloat16,
                                              space="PSUM", tag="tr")
                        nc.tensor.transpose(meanT_ps[:], rows[:, g0 + j, :],
                                            ident_f[:])
                        nc.scalar.activation(
                            meanT_stage[:, j * P: (j + 1) * P], meanT_ps[:],
                            mybir.ActivationFunctionType.Copy)
                    emit_mlp(meanT_stage, nmb, stage_out, g0)
                nc.scalar.dma_start(out_o[:, b0 * P: (b0 + ngb) * P],
                                    stage_out[:, : ngb * P])
    nc.compile()
    return nc


# ---------------------------------------------------------------------------

_CACHE = {}


def _get(key, fn):
    if key not in _CACHE:
        _CACHE[key] = fn()
    return _CACHE[key]


def kernel(x, edge_index, w_lead, b_lead, w1, b1, w2, b2):
    x = np.asarray(x, np.float32)
    assert x.shape == (N_NODES, H)

    ekey = hashlib.md5(np.asarray(edge_index).tobytes()).hexdigest()
    pp = _get(("pp", ekey), lambda: _preprocess(edge_index))

    K = pp["K"]
    inv = pp["inv"]
    inv_flat = inv.reshape(NCORES, NPC)

    xbf_ext = np.zeros((N_NODES + 1, H), bf16)
    xbf_ext[:N_NODES] = x.astype(bf16)
    xpad = np.zeros((NPAD, H), np.float32)
    xpad[:N_NODES] = x

    wrepc = np.ascontiguousarray(
        np.asarray(w_lead, np.float32).reshape(H, 1))
    blead = np.full((1, 1), np.float32(b_lead), np.float32)
    w1f = np.ascontiguousarray(np.asarray(w1, np.float32).astype(bf16))
    b1c = np.ascontiguousarray(np.asarray(b1, np.float32).reshape(H, 1))
    w2f = np.ascontiguousarray(np.asarray(w2, np.float32).astype(bf16))
    b2c = np.ascontiguousarray(np.asarray(b2, np.float32).reshape(C, 1))

    # ---- launch 1 ----------------------------------------------------------
    nc1 = _get(("l1", ekey), lambda: _build_l1(pp))
    in_maps = []
    for k in range(NCORES):
        # ET[k]: [P, K*H] partition-major slot table (slot = c*128+p)
        et = xbf_ext[pp["srcidx"][k]].reshape(K, P, H).transpose(1, 0, 2)
        et = np.ascontiguousarray(et).reshape(P, K * H)
        # xfT: [H, NPC] owned-node features transposed (f32, for logits)
        xf = np.where((inv[k] >= 0)[:, :, None],
                      xpad[np.maximum(inv[k], 0)], 0.0)  # [BPC, P, H]
        xfT = np.ascontiguousarray(
            xf.reshape(NPC, H).T.astype(np.float32))
        in_maps.append({
            "et": et,
            "xfT": xfT,
            "recip": pp["recip"][k],
            "wrepc": wrepc,
            "blead": blead,
        })
    r1 = run_bass_kernel_spmd(nc1, in_maps, core_ids=CORES)

    logits_full = np.zeros(NPAD, np.float32)
    means_full = np.zeros((NPAD, H), bf16)
    for k in range(NCORES):
        lg = r1.results[k]["logits_o"].reshape(NPC)     # [b*128+p]
        mn = r1.results[k]["means_o"].reshape(P, BPC, H)
        mf = inv_flat[k] >= 0
        ids = inv_flat[k][mf]
        logits_full[ids] = lg[mf]
        m = inv[k] >= 0                                 # [BPC, P]
        means_full[inv[k][m]] = mn.transpose(1, 0, 2)[m]

    # ---- launch 2: election + mean gather + MLP ----------------------------
    nc2 = _get(("l2", ekey), lambda: _build_l2(pp))
    es = pp["elog_src"]
    msrc = pp["msrc"]
    MSCOLS = pp["MSCOLS"]
    logits_ext = np.full(NPAD + 1, NEG, np.float32)
    logits_ext[:NPAD] = logits_full
    in_maps2 = []
    zpad = np.zeros((P, 1), np.float32)
    for k in range(NCORES):
        if pp["SWR"] > 0:
            ep = np.ascontiguousarray(
                np.where(es[k] >= 0, logits_full[np.maximum(es[k], 0)],
                         NEG).astype(np.float32))
            sp1k = pp["srcp1"][k]
        else:
            ep, sp1k = zpad, zpad
        if MSCOLS > 0:
            melog = np.where(msrc[k] < N_NODES, logits_ext[msrc[k]],
                             NEG).astype(np.float32)
            mset = means_full[msrc[k]].reshape(P, MSCOLS * H)
        else:
            melog = zpad
            mset = np.zeros((P, H), bf16)
        in_maps2.append({
            "epad": ep,
            "srcp1": sp1k,
            "meansfull": means_full,
            "melog": np.ascontiguousarray(melog),
            "mset": np.ascontiguousarray(mset),
            "w1": w1f,
            "b1": b1c,
            "w2": w2f,
            "b2c": b2c,
        })
    r2 = run_bass_kernel_spmd(nc2, in_maps2, core_ids=CORES)

    out = np.zeros((N_NODES, C), np.float32)
    for k in range(NCORES):
        g = r2.results[k]["out_o"].reshape(C, BPC, P).astype(np.float32)
        node_rows = g.transpose(1, 2, 0).reshape(NPC, C)
        m = inv_flat[k] >= 0
        out[inv_flat[k][m]] = node_rows[m]
    return out


# revision 38
# speedup vs baseline: 1.1649x; 1.0080x over previous
"""Trainium2 Bass kernel for nn_DecentralizedCoordinator (GNN message passing).

Strategy (8 NeuronCores, SPMD), v11:
- Nodes degree-sorted and round-robin sharded: global in-degree rank r ->
  core r%8, block (r//8)//128, slot (r//8)%128. Each block then holds
  same-degree nodes, so the edge-source feature table ET needs exactly
  Kb = max-indeg-in-block identity-lhsT matmul columns per block: no
  one-hot tail region at all, ~1.7% padding, identical layout per core.
- Launch 1 (DMA-bound, ~120us): leader logits via w_lead-stationary
  matmuls over an f32 xfT stream interleaved with the ET windows;
  segment sums via identity-lhsT matmul accumulation over the streamed
  bf16 ET; mean = sums * recip(indeg) on DVE -> means (bf16, dst-major)
  + logits (f32) to DRAM.
- Host between launches (index-pattern reshuffles only): assemble the
  means_full[node] table and the ragged per-chunk padded election layout
  epad/srcp1 from logits.
- Launch 2 (GpSimd descriptor-gen bound, ~162us): leader election with
  the exact reference tie-break (seg-max logits, then max src id) on
  DVE; back-to-back indirect-DMA gathers of each dst's LEADER'S MEAN row
  (98 x 128-row gathers, the Q7 descriptor-generation floor); then
  transpose + MLP (w1 -> gelu+b1 -> w2 -> +b2) on the gathered rows,
  with PE/Scalar work hidden under the gathers.
  (NB_IND < 98 enables an experimental masked-sum path for low-degree
  blocks; it is disabled -- concurrent streams starve the Q7 descriptor
  drain and showed rare nondeterministic corruption on cold runs.)

Host only shards/permutes/reshuffles by precomputed index patterns; every
operation on runtime values (logits, sums, means, MLP, comparisons,
election, the leader gather) is on device.
"""
import hashlib
import sys

import numpy as np
import ml_dtypes

sys.path.insert(0, "/opt/trn_rl_repo")

import concourse.bass as bass
import concourse.tile as tile
from concourse import bacc, mybir
from concourse.bass_utils import run_bass_kernel_spmd
from concourse.masks import make_identity

dt = mybir.dt
bf16 = ml_dtypes.bfloat16

P = 128
NCORES = 8
BPC = 98                 # dst blocks per core
NPC = BPC * P            # 12544 nodes per core
NPAD = NCORES * NPC      # 100352 padded node count
N_NODES = 100000
H = 128
C = 128
NEG = -3.0e38
CH = 16                  # blocks per launch-2 indirect chunk
NB_IND = 98              # blocks on the indirect-gather path (rest: masked)
MSW = 72                 # masked-window column budget

CORES = list(range(NCORES))


def _preprocess(edge_index):
    row = np.asarray(edge_index[0], np.int64)
    col = np.asarray(edge_index[1], np.int64)
    E = len(row)

    indeg = np.bincount(col, minlength=N_NODES)

    # degree-sorted round-robin assignment: rank r -> (r%8, (r//8)//128,
    # (r//8)%128)
    order = np.argsort(-indeg, kind="stable")          # rank -> node
    rr = np.arange(N_NODES)
    kk_of_rank = rr % NCORES
    pos = rr // NCORES
    bb_of_rank = pos // P
    pp_of_rank = pos % P
    node2kbp = np.zeros((N_NODES, 3), np.int64)
    node2kbp[order, 0] = kk_of_rank
    node2kbp[order, 1] = bb_of_rank
    node2kbp[order, 2] = pp_of_rank
    inv = np.full((NCORES, BPC, P), -1, np.int64)
    inv[kk_of_rank, bb_of_rank, pp_of_rank] = order

    # per-block column count (shared across cores): max indeg in the
    # 1024-rank group = indeg of its first (sorted desc)
    Kb = np.zeros(BPC, np.int64)
    for b in range(BPC):
        r0 = b * NCORES * P
        Kb[b] = indeg[order[r0]] if r0 < N_NODES else 0
    cb = np.concatenate([[0], np.cumsum(Kb)])          # col offset per block
    K = int(cb[-1])

    # edges grouped by dst, rank within dst
    dorder = np.argsort(col, kind="stable")
    row_d = row[dorder]
    col_d = col[dorder]
    dst_starts = np.concatenate([[0], np.cumsum(indeg)])
    ranks = np.arange(E) - dst_starts[col_d]

    kk = node2kbp[col_d, 0]
    bb = node2kbp[col_d, 1]
    ppos = node2kbp[col_d, 2]

    srcidx = np.full((NCORES, K * P), N_NODES, np.int64)
    srcidx[kk, (cb[bb] + ranks) * P + ppos] = row_d

    # recip of true in-degree per owned node, [NCORES, P, BPC]
    cnt = np.where(inv >= 0, indeg[np.maximum(inv, 0)], 0.0)   # [NC,BPC,P]
    recip = np.ascontiguousarray(
        (1.0 / np.maximum(cnt, 1.0)).transpose(0, 2, 1)).astype(np.float32)

    # ET stream windows: whole blocks, ~56 cols each
    windows = []           # (b_start, b_end, col0, ncols)
    b = 0
    while b < BPC:
        b1 = b
        ncols = 0
        while b1 < BPC and (ncols + Kb[b1] <= 56 or b1 == b):
            ncols += int(Kb[b1])
            b1 += 1
        windows.append((b, b1, int(cb[b]), ncols))
        b = b1

    # ragged election layout (indirect blocks only): chunk c covers CH
    # blocks, width Wc = max ext-degree (indeg+1) in chunk
    chunks = []            # (b0, ngb, Wc, off)
    off = 0
    b0 = 0
    while b0 < NB_IND:
        ngb = min(4 if b0 == 0 else CH, NB_IND - b0)
        r0 = b0 * NCORES * P
        Wc = int(indeg[order[r0]]) + 1 if r0 < N_NODES else 1
        chunks.append((b0, ngb, Wc, off))
        off += ngb * Wc
        b0 += ngb
    SWR = off

    WMAX = max([w for (_, _, w, _) in chunks], default=1)
    elog_src = np.full((NCORES, P, SWR), -1, np.int64)
    for (b0, ngb, Wc, coff) in chunks:
        for bi in range(ngb):
            b = b0 + bi
            base = coff + bi * Wc
            for k in range(NCORES):
                for p in range(P):
                    d = int(inv[k, b, p])
                    if d < 0:
                        continue
                    s0, s1 = int(dst_starts[d]), int(dst_starts[d + 1])
                    m = s1 - s0
                    elog_src[k, p, base] = d
                    if m > 0:
                        elog_src[k, p, base + 1: base + 1 + m] = row_d[s0:s1]
    srcp1 = np.where(elog_src >= 0, elog_src + 1, 0).astype(np.float32)

    # ---- masked-sum side (blocks NB_IND..BPC) -------------------------
    # per-dst dedup candidate list: [self] + unique in-edge srcs != self
    srt = np.lexsort((row_d, col_d))
    c2, r2 = col_d[srt], row_d[srt]
    uniq = np.ones(E, bool)
    uniq[1:] = (c2[1:] != c2[:-1]) | (r2[1:] != r2[:-1])
    uniq &= (r2 != c2)
    cu, ru = c2[uniq], r2[uniq]
    dl = np.bincount(cu, minlength=N_NODES)            # dedup in-deg (no self)
    du_starts = np.concatenate([[0], np.cumsum(dl)])

    # per-block candidate width (shared across cores)
    wd = np.zeros(BPC, np.int64)
    for b in range(NB_IND, BPC):
        r0, r1 = b * NCORES * P, min((b + 1) * NCORES * P, N_NODES)
        wd[b] = (dl[order[r0:r1]].max() + 1) if r1 > r0 else 1

    # masked windows: blocks with uniform padded width Kw, nblk*Kw <= MSW
    mwin = []              # (b0, nblk, Kw, col0)
    b = NB_IND
    mc = 0
    while b < BPC:
        b1 = b
        Kw = int(wd[b])
        while b1 < BPC:
            nKw = max(Kw, int(wd[b1]))
            if ((b1 - b + 1) * nKw > MSW or b1 - b >= 12) and b1 > b:
                break
            Kw = nKw
            b1 += 1
        mwin.append((b, b1 - b, Kw, mc))
        mc += (b1 - b) * Kw
        b = b1
    MSCOLS = mc

    msrc = np.full((NCORES, P, MSCOLS), N_NODES, np.int64)
    for (b0m, nblk, Kw, col0m) in mwin:
        for bi in range(nblk):
            b = b0m + bi
            base = col0m + bi * Kw
            for k in range(NCORES):
                for p in range(P):
                    d = int(inv[k, b, p])
                    if d < 0:
                        continue
                    msrc[k, p, base] = d
                    s0, s1 = int(du_starts[d]), int(du_starts[d + 1])
                    m = s1 - s0
                    if m > 0:
                        msrc[k, p, base + 1: base + 1 + m] = ru[s0:s1]

    return dict(
        Kb=Kb, cb=cb, K=K, windows=windows,
        srcidx=srcidx, recip=recip,
        chunks=chunks, SWR=SWR, WMAX=WMAX, elog_src=elog_src, srcp1=srcp1,
        mwin=mwin, MSCOLS=MSCOLS, msrc=msrc,
        node2kbp=node2kbp, inv=inv,
    )


# ---------------------------------------------------------------------------
# launch 1: logits + segment sums + mean
# ---------------------------------------------------------------------------

def _build_l1(pp):
    Kb = pp["Kb"]
    cb = pp["cb"]
    K = pp["K"]
    windows = pp["windows"]

    nc = bacc.Bacc("TRN2", target_bir_lowering=False, debug=False,
                   num_devices=NCORES)
    et_d = nc.dram_tensor("et", [P, K * H], dt.bfloat16,
                          kind="ExternalInput")
    xfT_d = nc.dram_tensor("xfT", [P, NPC], dt.float32,
                           kind="ExternalInput")
    recip_d = nc.dram_tensor("recip", [P, BPC], dt.float32,
                             kind="ExternalInput")
    wrepc_d = nc.dram_tensor("wrepc", [H, 1], dt.float32,
                             kind="ExternalInput")
    blead_d = nc.dram_tensor("blead", [1, 1], dt.float32,
                             kind="ExternalInput")

    logits_o = nc.dram_tensor("logits_o", [1, NPC], dt.float32,
                              kind="ExternalOutput")
    means_o = nc.dram_tensor("means_o", [P, BPC * H], dt.bfloat16,
                             kind="ExternalOutput")

    SB = 14                                        # blocks per means stage

    with tile.TileContext(nc) as tc:
        with (
            tc.tile_pool(name="const", bufs=1) as cp,
            tc.tile_pool(name="g", bufs=5) as gp,
            tc.tile_pool(name="stage", bufs=2) as stp,
            tc.tile_pool(name="sums_ps", bufs=4, space="PSUM") as sums_pp,
            tc.tile_pool(name="lg_ps", bufs=2, space="PSUM") as lg_pp,
        ):
            recip_t = cp.tile([P, BPC], dt.float32)
            nc.scalar.dma_start(recip_t[:], recip_d[:, :])
            wrepc_t = cp.tile([H, 1], dt.float32)
            nc.scalar.dma_start(wrepc_t[:], wrepc_d[:, :])
            blead_t = cp.tile([1, 1], dt.float32)
            nc.scalar.dma_start(blead_t[:], blead_d[:, :])
            xfT_t = cp.tile([P, NPC], dt.float32)
            logits_sb = cp.tile([1, NPC], dt.float32)
            ident_f = cp.tile([P, P], dt.bfloat16)
            make_identity(nc, ident_f[:])

            stage_out = None
            for (bw0, bw1, col0, ncols) in windows:
                G = gp.tile([P, 56 * H], dt.bfloat16, tag="g")
                if ncols > 0:
                    nc.sync.dma_start(G[:, : ncols * H],
                                      et_d[:, col0 * H: (col0 + ncols) * H])
                q0w, q1w = bw0 * P, bw1 * P
                nc.sync.dma_start(xfT_t[:, q0w: q1w], xfT_d[:, q0w: q1w])
                for b in range(bw0, bw1):
                    sj = b % SB
                    if sj == 0:
                        stage_out = stp.tile([P, SB * H], dt.bfloat16,
                                             tag="st")
                    nb = int(Kb[b])
                    c0 = int(cb[b]) - col0
                    sums_ps = sums_pp.tile([P, H], dt.float32, space="PSUM",
                                           tag="sums")
                    if nb == 0:
                        nc.vector.memset(sums_ps[:], 0.0)
                    for j in range(nb):
                        nc.tensor.matmul(
                            out=sums_ps[:], lhsT=ident_f[:],
                            rhs=G[:, (c0 + j) * H: (c0 + j + 1) * H],
                            start=(j == 0), stop=(j == nb - 1))
                    nc.vector.tensor_scalar_mul(
                        stage_out[:, sj * H: (sj + 1) * H], sums_ps[:],
                        recip_t[:, b: b + 1])
                    if sj == SB - 1 or b == BPC - 1:
                        b0s = b - sj
                        nc.sync.dma_start(
                            means_o[:, b0s * H: (b + 1) * H],
                            stage_out[:, : (sj + 1) * H])
                # logits for this window's nodes: w_lead-stationary chunks
                for q0 in range(q0w, q1w, 448):
                    nq = min(448, q1w - q0)
                    lg_ps = lg_pp.tile([1, 448], dt.float32, space="PSUM",
                                       tag="lg")
                    nc.tensor.matmul(out=lg_ps[:, :nq], lhsT=wrepc_t[:],
                                     rhs=xfT_t[:, q0: q0 + nq],
                                     start=True, stop=True)
                    nc.scalar.activation(
                        logits_sb[:, q0: q0 + nq], lg_ps[:, :nq],
                        mybir.ActivationFunctionType.Identity,
                        bias=blead_t[:, :1])
            nc.sync.dma_start(logits_o[:, :], logits_sb[:])
    nc.compile()
    return nc


# ---------------------------------------------------------------------------
# launch 2: leader election + mean gather + MLP
# ---------------------------------------------------------------------------

def _view3(t, ngb, wc):
    """[P, ngb, wc] strided view of a [P, >=ngb*wc] tile."""
    a = t[:]
    return bass.AP(a.tensor, a.offset, [a.ap[0], [wc, ngb], [1, wc]])


def _build_l2(pp):
    chunks = pp["chunks"]
    SWR = pp["SWR"]
    WMAX = pp["WMAX"]
    mwin = pp["mwin"]
    MSCOLS = pp["MSCOLS"]

    nc = bacc.Bacc("TRN2", target_bir_lowering=False, debug=False,
                   num_devices=NCORES)
    SWR1 = max(SWR, 1)
    ep_d = nc.dram_tensor("epad", [P, SWR1], dt.float32,
                          kind="ExternalInput")
    sp1_d = nc.dram_tensor("srcp1", [P, SWR1], dt.float32,
                           kind="ExternalInput")
    means_d = nc.dram_tensor("meansfull", [NPAD, H], dt.bfloat16,
                             kind="ExternalInput")
    MSC1 = max(MSCOLS, 1)
    melog_d = nc.dram_tensor("melog", [P, MSC1], dt.float32,
                             kind="ExternalInput")
    mset_d = nc.dram_tensor("mset", [P, MSC1 * H], dt.bfloat16,
                            kind="ExternalInput")
    w1_d = nc.dram_tensor("w1", [H, H], dt.bfloat16, kind="ExternalInput")
    b1_d = nc.dram_tensor("b1", [P, 1], dt.float32, kind="ExternalInput")
    w2_d = nc.dram_tensor("w2", [H, C], dt.bfloat16, kind="ExternalInput")
    b2_d = nc.dram_tensor("b2c", [C, 1], dt.float32, kind="ExternalInput")
    out_o = nc.dram_tensor("out_o", [C, NPC], dt.bfloat16,
                           kind="ExternalOutput")

    MB = 4                                          # blocks per MLP group

    with tile.TileContext(nc) as tc:
        with (
            tc.tile_pool(name="const", bufs=1) as cp,
            tc.tile_pool(name="ein", bufs=2) as eip,
            tc.tile_pool(name="ework", bufs=2) as ewp,
            tc.tile_pool(name="lead", bufs=8) as ldp,
            tc.tile_pool(name="rows", bufs=8) as rp_,
            tc.tile_pool(name="win", bufs=max(len(mwin), 1)) as wnp,
            tc.tile_pool(name="gms", bufs=4) as gmp,
            tc.tile_pool(name="mw", bufs=3) as mwp,
            tc.tile_pool(name="mst", bufs=2) as mp,
            tc.tile_pool(name="ost", bufs=2) as stp,
            tc.tile_pool(name="ostm", bufs=2) as stp2,
            tc.tile_pool(name="tr_ps", bufs=2, space="PSUM") as tr_pp,
            tc.tile_pool(name="sel_ps", bufs=3, space="PSUM") as sel_pp,
            tc.tile_pool(name="mlp_ps", bufs=2, space="PSUM") as mlp_pp,
        ):
            w1_t = cp.tile([H, H], dt.bfloat16)
            nc.scalar.dma_start(w1_t[:], w1_d[:, :])
            b1_t = cp.tile([P, 1], dt.float32)
            nc.scalar.dma_start(b1_t[:], b1_d[:, :])
            w2_t = cp.tile([H, C], dt.bfloat16)
            nc.scalar.dma_start(w2_t[:], w2_d[:, :])
            b2_t = cp.tile([C, 1], dt.float32)
            nc.scalar.dma_start(b2_t[:], b2_d[:, :])
            ident_f = cp.tile([P, P], dt.bfloat16)
            make_identity(nc, ident_f[:])
            melog_t = cp.tile([P, MSC1], dt.float32)
            if MSCOLS > 0:
                nc.scalar.dma_start(melog_t[:], melog_d[:, :])

            def emit_mlp(meanT_stage, nmb, stage_out, g0):
                hpre_ps = mlp_pp.tile([P, MB * H], dt.float32,
                                      space="PSUM", tag="mlp")
                nc.tensor.matmul(out=hpre_ps[:, : nmb * H], lhsT=w1_t[:],
                                 rhs=meanT_stage[:, : nmb * P],
                                 start=True, stop=True)
                hT_stage = mp.tile([P, MB * H], dt.bfloat16, tag="hT")
                nc.scalar.activation(hT_stage[:, : nmb * H],
                                     hpre_ps[:, : nmb * H],
                                     mybir.ActivationFunctionType.Gelu,
                                     bias=b1_t[:, :1])
                rep_ps = mlp_pp.tile([P, MB * P], dt.float32,
                                     space="PSUM", tag="mlp")
                nc.tensor.matmul(out=rep_ps[:, : nmb * P], lhsT=w2_t[:],
                                 rhs=hT_stage[:, : nmb * H],
                                 start=True, stop=True)
                nc.scalar.activation(
                    stage_out[:, g0 * P: (g0 + nmb) * P],
                    rep_ps[:, : nmb * P],
                    mybir.ActivationFunctionType.Identity,
                    bias=b2_t[:, :1])

            # phase 1: indirect-side elections (DVE) -> per-chunk leadi
            leadis = []
            for (b0, ngb, Wc, coff) in chunks:
                n = ngb * Wc
                ep = eip.tile([P, CH * WMAX], dt.float32, tag="ep")
                nc.sync.dma_start(ep[:, :n], ep_d[:, coff: coff + n])
                epv = _view3(ep, ngb, Wc)
                sp1 = eip.tile([P, CH * WMAX], dt.float32, tag="sp1")
                nc.sync.dma_start(sp1[:, :n], sp1_d[:, coff: coff + n])
                sp1v = _view3(sp1, ngb, Wc)

                sm = ewp.tile([P, CH], dt.float32, tag="sm")
                nc.vector.reduce_max(out=sm[:, :ngb], in_=epv,
                                     axis=mybir.AxisListType.X)
                mask = ewp.tile([P, CH * WMAX], dt.float32, tag="mask")
                maskv = _view3(mask, ngb, Wc)
                a = sm[:]
                sm_b = bass.AP(a.tensor, a.offset,
                               [a.ap[0], [1, ngb], [0, Wc]])
                nc.vector.tensor_tensor(out=maskv, in0=epv, in1=sm_b,
                                        op=mybir.AluOpType.is_equal)
                cand = ewp.tile([P, CH * WMAX], dt.float32, tag="cand")
                candv = _view3(cand, ngb, Wc)
                nc.vector.tensor_tensor(out=candv, in0=maskv, in1=sp1v,
                                        op=mybir.AluOpType.mult)
                lp1 = ewp.tile([P, CH], dt.float32, tag="lp1")
                nc.vector.reduce_max(out=lp1[:, :ngb], in_=candv,
                                     axis=mybir.AxisListType.X)
                leadf = ewp.tile([P, CH], dt.float32, tag="leadf")
                nc.vector.tensor_scalar(
                    out=leadf[:, :ngb], in0=lp1[:, :ngb], scalar1=-1.0,
                    scalar2=0.0,
                    op0=mybir.AluOpType.add, op1=mybir.AluOpType.max)
                leadi = ldp.tile([P, CH], dt.int32, tag="leadi")
                nc.vector.tensor_copy(leadi[:, :ngb], leadf[:, :ngb])
                leadis.append(leadi)

            # phase 2: issue all indirect gathers (GpSimd self-paces; they
            # drain after the mset stream quiesces)
            rows_l = []
            for ci, (b0, ngb, Wc, coff) in enumerate(chunks):
                leadi = leadis[ci]
                rows = rp_.tile([P, CH, H], dt.bfloat16, tag="rows")
                for j in range(ngb):
                    nc.gpsimd.indirect_dma_start(
                        out=rows[:, j, :],
                        out_offset=None,
                        in_=means_d[:, :],
                        in_offset=bass.IndirectOffsetOnAxis(
                            ap=leadi[:, j: j + 1], axis=0),
                    )
                rows_l.append(rows)

            # phase 3: masked-side winner masks (DVE)
            wins = []
            for (b0m, nblk, Kw, col0m) in mwin:
                nmc = nblk * Kw
                smx = ewp.tile([P, MSW], dt.float32, tag="smx")
                ml = melog_t[:, col0m: col0m + nmc]
                lv = bass.AP(ml.tensor, ml.offset,
                             [ml.ap[0], [Kw, nblk], [1, Kw]])
                nc.vector.reduce_max(out=smx[:, :nblk], in_=lv,
                                     axis=mybir.AxisListType.X)
                win = wnp.tile([P, MSW], dt.bfloat16, tag="win")
                winv = _view3(win, nblk, Kw)
                a = smx[:]
                smx_b = bass.AP(a.tensor, a.offset,
                                [a.ap[0], [1, nblk], [0, Kw]])
                nc.vector.tensor_tensor(out=winv, in0=lv, in1=smx_b,
                                        op=mybir.AluOpType.is_equal)
                wins.append(win)

            # phase 4: mset stream + select + MLP
            gp_elems, dve_elems = 0, 1
            for wi, (b0m, nblk, Kw, col0m) in enumerate(mwin):
                nmc = nblk * Kw
                G = gmp.tile([P, MSW * H], dt.bfloat16, tag="gms")
                nc.sync.dma_start(G[:, : nmc * H],
                                  mset_d[:, col0m * H: (col0m + nmc) * H])
                win = wins[wi]
                mw = mwp.tile([P, MSW * H], dt.bfloat16, tag="mw")
                ga = G[:, : nmc * H]
                gv = bass.AP(ga.tensor, ga.offset,
                             [ga.ap[0], [H, nmc], [1, H]])
                wa = win[:, : nmc]
                wv = bass.AP(wa.tensor, wa.offset,
                             [wa.ap[0], [1, nmc], [0, H]])
                ma = mw[:, : nmc * H]
                mv = bass.AP(ma.tensor, ma.offset,
                             [ma.ap[0], [H, nmc], [1, H]])
                if gp_elems * 2 < dve_elems:
                    eng = nc.gpsimd
                    gp_elems += nmc
                else:
                    eng = nc.vector
                    dve_elems += nmc
                eng.tensor_tensor(out=mv, in0=gv, in1=wv,
                                  op=mybir.AluOpType.mult)

                stage_out = stp2.tile([P, 12 * P], dt.bfloat16, tag="stm")
                meanT_stage = None
                for bi in range(nblk):
                    gj = bi % MB
                    if gj == 0:
                        meanT_stage = mp.tile([P, MB * P], dt.bfloat16,
                                              tag="meanT")
                    sel_ps = sel_pp.tile([P, 512], dt.float32, space="PSUM",
                                         tag="sel")
                    for j in range(Kw):
                        c = bi * Kw + j
                        nc.tensor.matmul(out=sel_ps[:, :P],
                                         lhsT=mw[:, c * H: (c + 1) * H],
                                         rhs=ident_f[:],
                                         start=(j == 0), stop=(j == Kw - 1))
                    nc.scalar.activation(
                        meanT_stage[:, gj * P: (gj + 1) * P], sel_ps[:, :P],
                        mybir.ActivationFunctionType.Copy)
                    if gj == MB - 1 or bi == nblk - 1:
                        emit_mlp(meanT_stage, gj + 1, stage_out, bi - gj)
                nc.scalar.dma_start(out_o[:, b0m * P: (b0m + nblk) * P],
                                    stage_out[:, : nblk * P])

            # phase 5: indirect-side MLP
            for ci, (b0, ngb, Wc, coff) in enumerate(chunks):
                rows = rows_l[ci]
                stage_out = stp.tile([P, CH * P], dt.bfloat16, tag="st")
                for g0 in range(0, ngb, MB):
                    nmb = min(MB, ngb - g0)
                    meanT_stage = mp.tile([P, MB * P], dt.bfloat16,
                                          tag="meanT")
                    for j in range(nmb):
                        meanT_ps = tr_pp.tile([P, P], dt.bfloat16,
                                              space="PSUM", tag="tr")
                        nc.tensor.transpose(meanT_ps[:], rows[:, g0 + j, :],
                                            ident_f[:])
                        nc.scalar.activation(
                            meanT_stage[:, j * P: (j + 1) * P], meanT_ps[:],
                            mybir.ActivationFunctionType.Copy)
                    emit_mlp(meanT_stage, nmb, stage_out, g0)
                nc.scalar.dma_start(out_o[:, b0 * P: (b0 + ngb) * P],
                                    stage_out[:, : ngb * P])
    nc.compile()
    return nc


# ---------------------------------------------------------------------------

_CACHE = {}


def _get(key, fn):
    if key not in _CACHE:
        _CACHE[key] = fn()
    return _CACHE[key]


def kernel(x, edge_index, w_lead, b_lead, w1, b1, w2, b2):
    x = np.asarray(x, np.float32)
    assert x.shape == (N_NODES, H)

    ekey = hashlib.md5(np.asarray(edge_index).tobytes()).hexdigest()
    pp = _get(("pp", ekey), lambda: _preprocess(edge_index))

    K = pp["K"]
    inv = pp["inv"]
    inv_flat = inv.reshape(NCORES, NPC)

    xbf_ext = np.zeros((N_NODES + 1, H), bf16)
    xbf_ext[:N_NODES] = x.astype(bf16)
    xpad = np.zeros((NPAD, H), np.float32)
    xpad[:N_NODES] = x

    wrepc = np.ascontiguousarray(
        np.asarray(w_lead, np.float32).reshape(H, 1))
    blead = np.full((1, 1), np.float32(b_lead), np.float32)
    w1f = np.ascontiguousarray(np.asarray(w1, np.float32).astype(bf16))
    b1c = np.ascontiguousarray(np.asarray(b1, np.float32).reshape(H, 1))
    w2f = np.ascontiguousarray(np.asarray(w2, np.float32).astype(bf16))
    b2c = np.ascontiguousarray(np.asarray(b2, np.float32).reshape(C, 1))

    # ---- launch 1 ----------------------------------------------------------
    nc1 = _get(("l1", ekey), lambda: _build_l1(pp))
    in_maps = []
    for k in range(NCORES):
        # ET[k]: [P, K*H] partition-major slot table (slot = c*128+p)
        et = xbf_ext[pp["srcidx"][k]].reshape(K, P, H).transpose(1, 0, 2)
        et = np.ascontiguousarray(et).reshape(P, K * H)
        # xfT: [H, NPC] owned-node features transposed (f32, for logits)
        xf = np.where((inv[k] >= 0)[:, :, None],
                      xpad[np.maximum(inv[k], 0)], 0.0)  # [BPC, P, H]
        xfT = np.ascontiguousarray(
            xf.reshape(NPC, H).T.astype(np.float32))
        in_maps.append({
            "et": et,
            "xfT": xfT,
            "recip": pp["recip"][k],
            "wrepc": wrepc,
            "blead": blead,
        })
    r1 = run_bass_kernel_spmd(nc1, in_maps, core_ids=CORES)

    logits_full = np.zeros(NPAD, np.float32)
    means_full = np.zeros((NPAD, H), bf16)
    for k in range(NCORES):
        lg = r1.results[k]["logits_o"].reshape(NPC)     # [b*128+p]
        mn = r1.results[k]["means_o"].reshape(P, BPC, H)
        mf = inv_flat[k] >= 0
        ids = inv_flat[k][mf]
        logits_full[ids] = lg[mf]
        m = inv[k] >= 0                                 # [BPC, P]
        means_full[inv[k][m]] = mn.transpose(1, 0, 2)[m]

    # ---- launch 2: election + mean gather + MLP ----------------------------
    nc2 = _get(("l2", ekey), lambda: _build_l2(pp))
    es = pp["elog_src"]
    msrc = pp["msrc"]
    MSCOLS = pp["MSCOLS"]
    logits_ext = np.full(NPAD + 1, NEG, np.float32)
    logits_ext[:NPAD] = logits_full
    in_maps2 = []
    zpad = np.zeros((P, 1), np.float32)
    for k in range(NCORES):
        if pp["SWR"] > 0:
            ep = np.ascontiguousarray(
                np.where(es[k] >= 0, logits_full[np.maximum(es[k], 0)],
                         NEG).astype(np.float32))
            sp1k = pp["srcp1"][k]
        else:
            ep, sp1k = zpad, zpad
        if MSCOLS > 0:
            melog = np.where(msrc[k] < N_NODES, logits_ext[msrc[k]],
                             NEG).astype(np.float32)
            mset = means_full[msrc[k]].reshape(P, MSCOLS * H)
        else:
            melog = zpad
            mset = np.zeros((P, H), bf16)
        in_maps2.append({
            "epad": ep,
            "srcp1": sp1k,
            "meansfull": means_full,
            "melog": np.ascontiguousarray(melog),
            "mset": np.ascontiguousarray(mset),
            "w1": w1f,
            "b1": b1c,
            "w2": w2f,
            "b2c": b2c,
        })
    r2 = run_bass_kernel_spmd(nc2, in_maps2, core_ids=CORES)

    out = np.zeros((N_NODES, C), np.float32)
    for k in range(NCORES):
        g = r2.results[k]["out_o"].reshape(C, BPC, P).astype(np.float32)
        node_rows = g.transpose(1, 2, 0).reshape(NPC, C)
        m = inv_flat[k] >= 0
        out[inv_flat[k][m]] = node_rows[m]
    return out


# revision 39
# speedup vs baseline: 1.2108x; 1.0394x over previous
"""Trainium2 Bass kernel for nn_DecentralizedCoordinator (GNN message passing).

Strategy (8 NeuronCores, SPMD), v11:
- Nodes degree-sorted and round-robin sharded: global in-degree rank r ->
  core r%8, block (r//8)//128, slot (r//8)%128. Each block then holds
  same-degree nodes, so the edge-source feature table ET needs exactly
  Kb = max-indeg-in-block identity-lhsT matmul columns per block: no
  one-hot tail region at all, ~1.7% padding, identical layout per core.
- Launch 1 (DMA-bound, ~120us): leader logits via w_lead-stationary
  matmuls over an f32 xfT stream interleaved with the ET windows;
  segment sums via identity-lhsT matmul accumulation over the streamed
  bf16 ET; mean = sums * recip(indeg) on DVE -> means (bf16, dst-major)
  + logits (f32) to DRAM.
- Host between launches (index-pattern reshuffles only): assemble the
  means_full[node] table and the ragged per-chunk padded election layout
  epad/srcp1 from logits.
- Launch 2 (GpSimd descriptor-gen bound, ~162us): leader election with
  the exact reference tie-break (seg-max logits, then max src id) on
  DVE; back-to-back indirect-DMA gathers of each dst's LEADER'S MEAN row
  (98 x 128-row gathers, the Q7 descriptor-generation floor); then
  transpose + MLP (w1 -> gelu+b1 -> w2 -> +b2) on the gathered rows,
  with PE/Scalar work hidden under the gathers.
  (NB_IND < 98 enables an experimental masked-sum path for low-degree
  blocks; it is disabled -- concurrent streams starve the Q7 descriptor
  drain and showed rare nondeterministic corruption on cold runs.)

Host only shards/permutes/reshuffles by precomputed index patterns; every
operation on runtime values (logits, sums, means, MLP, comparisons,
election, the leader gather) is on device.
"""
import hashlib
import sys

import numpy as np
import ml_dtypes

sys.path.insert(0, "/opt/trn_rl_repo")

import concourse.bass as bass
import concourse.tile as tile
from concourse import bacc, mybir
from concourse.bass_utils import run_bass_kernel_spmd
from concourse.masks import make_identity

dt = mybir.dt
bf16 = ml_dtypes.bfloat16

P = 128
NCORES = 8
BPC = 98                 # dst blocks per core
NPC = BPC * P            # 12544 nodes per core
NPAD = NCORES * NPC      # 100352 padded node count
N_NODES = 100000
H = 128
C = 128
NEG = -3.0e38
CH = 16                  # blocks per launch-2 indirect chunk
NB_IND = 98              # blocks on the indirect-gather path (rest: masked)
MSW = 72                 # masked-window column budget

CORES = list(range(NCORES))


def _preprocess(edge_index):
    row = np.asarray(edge_index[0], np.int64)
    col = np.asarray(edge_index[1], np.int64)
    E = len(row)

    indeg = np.bincount(col, minlength=N_NODES)

    # degree-sorted round-robin assignment: rank r -> (r%8, (r//8)//128,
    # (r//8)%128)
    order = np.argsort(-indeg, kind="stable")          # rank -> node
    rr = np.arange(N_NODES)
    kk_of_rank = rr % NCORES
    pos = rr // NCORES
    bb_of_rank = pos // P
    pp_of_rank = pos % P
    node2kbp = np.zeros((N_NODES, 3), np.int64)
    node2kbp[order, 0] = kk_of_rank
    node2kbp[order, 1] = bb_of_rank
    node2kbp[order, 2] = pp_of_rank
    inv = np.full((NCORES, BPC, P), -1, np.int64)
    inv[kk_of_rank, bb_of_rank, pp_of_rank] = order

    # per-block column count (shared across cores): max indeg in the
    # 1024-rank group = indeg of its first (sorted desc)
    Kb = np.zeros(BPC, np.int64)
    for b in range(BPC):
        r0 = b * NCORES * P
        Kb[b] = indeg[order[r0]] if r0 < N_NODES else 0
    cb = np.concatenate([[0], np.cumsum(Kb)])          # col offset per block
    K = int(cb[-1])

    # edges grouped by dst, rank within dst
    dorder = np.argsort(col, kind="stable")
    row_d = row[dorder]
    col_d = col[dorder]
    dst_starts = np.concatenate([[0], np.cumsum(indeg)])
    ranks = np.arange(E) - dst_starts[col_d]

    kk = node2kbp[col_d, 0]
    bb = node2kbp[col_d, 1]
    ppos = node2kbp[col_d, 2]

    srcidx = np.full((NCORES, K * P), N_NODES, np.int64)
    srcidx[kk, (cb[bb] + ranks) * P + ppos] = row_d

    # recip of true in-degree per owned node, [NCORES, P, BPC]
    cnt = np.where(inv >= 0, indeg[np.maximum(inv, 0)], 0.0)   # [NC,BPC,P]
    recip = np.ascontiguousarray(
        (1.0 / np.maximum(cnt, 1.0)).transpose(0, 2, 1)).astype(np.float32)

    # ET stream windows: whole blocks, ~56 cols each
    windows = []           # (b_start, b_end, col0, ncols)
    b = 0
    while b < BPC:
        b1 = b
        ncols = 0
        while b1 < BPC and (ncols + Kb[b1] <= 56 or b1 == b):
            ncols += int(Kb[b1])
            b1 += 1
        windows.append((b, b1, int(cb[b]), ncols))
        b = b1

    # ragged election layout (indirect blocks only): chunk c covers CH
    # blocks, width Wc = max ext-degree (indeg+1) in chunk
    chunks = []            # (b0, ngb, Wc, off)
    off = 0
    b0 = 0
    while b0 < NB_IND:
        ngb = min(4 if b0 == 0 else CH, NB_IND - b0)
        r0 = b0 * NCORES * P
        Wc = int(indeg[order[r0]]) + 1 if r0 < N_NODES else 1
        chunks.append((b0, ngb, Wc, off))
        off += ngb * Wc
        b0 += ngb
    SWR = off

    WMAX = max([w for (_, _, w, _) in chunks], default=1)
    elog_src = np.full((NCORES, P, SWR), -1, np.int64)
    for (b0, ngb, Wc, coff) in chunks:
        for bi in range(ngb):
            b = b0 + bi
            base = coff + bi * Wc
            for k in range(NCORES):
                for p in range(P):
                    d = int(inv[k, b, p])
                    if d < 0:
                        continue
                    s0, s1 = int(dst_starts[d]), int(dst_starts[d + 1])
                    m = s1 - s0
                    elog_src[k, p, base] = d
                    if m > 0:
                        elog_src[k, p, base + 1: base + 1 + m] = row_d[s0:s1]
    srcp1 = np.where(elog_src >= 0, elog_src + 1, 0).astype(np.float32)

    # ---- masked-sum side (blocks NB_IND..BPC) -------------------------
    # per-dst dedup candidate list: [self] + unique in-edge srcs != self
    srt = np.lexsort((row_d, col_d))
    c2, r2 = col_d[srt], row_d[srt]
    uniq = np.ones(E, bool)
    uniq[1:] = (c2[1:] != c2[:-1]) | (r2[1:] != r2[:-1])
    uniq &= (r2 != c2)
    cu, ru = c2[uniq], r2[uniq]
    dl = np.bincount(cu, minlength=N_NODES)            # dedup in-deg (no self)
    du_starts = np.concatenate([[0], np.cumsum(dl)])

    # per-block candidate width (shared across cores)
    wd = np.zeros(BPC, np.int64)
    for b in range(NB_IND, BPC):
        r0, r1 = b * NCORES * P, min((b + 1) * NCORES * P, N_NODES)
        wd[b] = (dl[order[r0:r1]].max() + 1) if r1 > r0 else 1

    # masked windows: blocks with uniform padded width Kw, nblk*Kw <= MSW
    mwin = []              # (b0, nblk, Kw, col0)
    b = NB_IND
    mc = 0
    while b < BPC:
        b1 = b
        Kw = int(wd[b])
        while b1 < BPC:
            nKw = max(Kw, int(wd[b1]))
            if ((b1 - b + 1) * nKw > MSW or b1 - b >= 12) and b1 > b:
                break
            Kw = nKw
            b1 += 1
        mwin.append((b, b1 - b, Kw, mc))
        mc += (b1 - b) * Kw
        b = b1
    MSCOLS = mc

    msrc = np.full((NCORES, P, MSCOLS), N_NODES, np.int64)
    for (b0m, nblk, Kw, col0m) in mwin:
        for bi in range(nblk):
            b = b0m + bi
            base = col0m + bi * Kw
            for k in range(NCORES):
                for p in range(P):
                    d = int(inv[k, b, p])
                    if d < 0:
                        continue
                    msrc[k, p, base] = d
                    s0, s1 = int(du_starts[d]), int(du_starts[d + 1])
                    m = s1 - s0
                    if m > 0:
                        msrc[k, p, base + 1: base + 1 + m] = ru[s0:s1]

    return dict(
        Kb=Kb, cb=cb, K=K, windows=windows,
        srcidx=srcidx, recip=recip,
        chunks=chunks, SWR=SWR, WMAX=WMAX, elog_src=elog_src, srcp1=srcp1,
        mwin=mwin, MSCOLS=MSCOLS, msrc=msrc,
        node2kbp=node2kbp, inv=inv,
    )


# ---------------------------------------------------------------------------
# launch 1: logits + segment sums + mean
# ---------------------------------------------------------------------------

def _build_l1(pp):
    Kb = pp["Kb"]
    cb = pp["cb"]
    K = pp["K"]
    windows = pp["windows"]

    nc = bacc.Bacc("TRN2", target_bir_lowering=False, debug=False,
                   num_devices=NCORES)
    et_d = nc.dram_tensor("et", [P, K * H], dt.bfloat16,
                          kind="ExternalInput")
    xfT_d = nc.dram_tensor("xfT", [P, NPC], dt.float32,
                           kind="ExternalInput")
    recip_d = nc.dram_tensor("recip", [P, BPC], dt.float32,
                             kind="ExternalInput")
    wrepc_d = nc.dram_tensor("wrepc", [H, 1], dt.float32,
                             kind="ExternalInput")
    blead_d = nc.dram_tensor("blead", [1, 1], dt.float32,
                             kind="ExternalInput")

    logits_o = nc.dram_tensor("logits_o", [1, NPC], dt.float32,
                              kind="ExternalOutput")
    means_o = nc.dram_tensor("means_o", [P, BPC * H], dt.bfloat16,
                             kind="ExternalOutput")

    SB = 14                                        # blocks per means stage

    with tile.TileContext(nc) as tc:
        with (
            tc.tile_pool(name="const", bufs=1) as cp,
            tc.tile_pool(name="g", bufs=5) as gp,
            tc.tile_pool(name="stage", bufs=2) as stp,
            tc.tile_pool(name="sums_ps", bufs=4, space="PSUM") as sums_pp,
            tc.tile_pool(name="lg_ps", bufs=2, space="PSUM") as lg_pp,
        ):
            recip_t = cp.tile([P, BPC], dt.float32)
            nc.scalar.dma_start(recip_t[:], recip_d[:, :])
            wrepc_t = cp.tile([H, 1], dt.float32)
            nc.scalar.dma_start(wrepc_t[:], wrepc_d[:, :])
            blead_t = cp.tile([1, 1], dt.float32)
            nc.scalar.dma_start(blead_t[:], blead_d[:, :])
            xfT_t = cp.tile([P, NPC], dt.float32)
            logits_sb = cp.tile([1, NPC], dt.float32)
            ident_f = cp.tile([P, P], dt.bfloat16)
            make_identity(nc, ident_f[:])

            stage_out = None
            for wi, (bw0, bw1, col0, ncols) in enumerate(windows):
                G = gp.tile([P, 56 * H], dt.bfloat16, tag="g")
                if ncols > 0:
                    ring = nc.scalar if wi % 2 else nc.sync
                    ring.dma_start(G[:, : ncols * H],
                                   et_d[:, col0 * H: (col0 + ncols) * H])
                q0w, q1w = bw0 * P, bw1 * P
                nc.sync.dma_start(xfT_t[:, q0w: q1w], xfT_d[:, q0w: q1w])
                for b in range(bw0, bw1):
                    sj = b % SB
                    if sj == 0:
                        stage_out = stp.tile([P, SB * H], dt.bfloat16,
                                             tag="st")
                    nb = int(Kb[b])
                    c0 = int(cb[b]) - col0
                    sums_ps = sums_pp.tile([P, H], dt.float32, space="PSUM",
                                           tag="sums")
                    if nb == 0:
                        nc.vector.memset(sums_ps[:], 0.0)
                    for j in range(nb):
                        nc.tensor.matmul(
                            out=sums_ps[:], lhsT=ident_f[:],
                            rhs=G[:, (c0 + j) * H: (c0 + j + 1) * H],
                            start=(j == 0), stop=(j == nb - 1))
                    nc.vector.tensor_scalar_mul(
                        stage_out[:, sj * H: (sj + 1) * H], sums_ps[:],
                        recip_t[:, b: b + 1])
                    if sj == SB - 1 or b == BPC - 1:
                        b0s = b - sj
                        nc.sync.dma_start(
                            means_o[:, b0s * H: (b + 1) * H],
                            stage_out[:, : (sj + 1) * H])
                # logits for this window's nodes: w_lead-stationary chunks
                for q0 in range(q0w, q1w, 448):
                    nq = min(448, q1w - q0)
                    lg_ps = lg_pp.tile([1, 448], dt.float32, space="PSUM",
                                       tag="lg")
                    nc.tensor.matmul(out=lg_ps[:, :nq], lhsT=wrepc_t[:],
                                     rhs=xfT_t[:, q0: q0 + nq],
                                     start=True, stop=True)
                    nc.scalar.activation(
                        logits_sb[:, q0: q0 + nq], lg_ps[:, :nq],
                        mybir.ActivationFunctionType.Identity,
                        bias=blead_t[:, :1])
            nc.sync.dma_start(logits_o[:, :], logits_sb[:])
    nc.compile()
    return nc


# ---------------------------------------------------------------------------
# launch 2: leader election + mean gather + MLP
# ---------------------------------------------------------------------------

def _view3(t, ngb, wc):
    """[P, ngb, wc] strided view of a [P, >=ngb*wc] tile."""
    a = t[:]
    return bass.AP(a.tensor, a.offset, [a.ap[0], [wc, ngb], [1, wc]])


def _build_l2(pp):
    chunks = pp["chunks"]
    SWR = pp["SWR"]
    WMAX = pp["WMAX"]
    mwin = pp["mwin"]
    MSCOLS = pp["MSCOLS"]

    nc = bacc.Bacc("TRN2", target_bir_lowering=False, debug=False,
                   num_devices=NCORES)
    SWR1 = max(SWR, 1)
    ep_d = nc.dram_tensor("epad", [P, SWR1], dt.float32,
                          kind="ExternalInput")
    sp1_d = nc.dram_tensor("srcp1", [P, SWR1], dt.float32,
                           kind="ExternalInput")
    means_d = nc.dram_tensor("meansfull", [NPAD, H], dt.bfloat16,
                             kind="ExternalInput")
    MSC1 = max(MSCOLS, 1)
    melog_d = nc.dram_tensor("melog", [P, MSC1], dt.float32,
                             kind="ExternalInput")
    mset_d = nc.dram_tensor("mset", [P, MSC1 * H], dt.bfloat16,
                            kind="ExternalInput")
    w1_d = nc.dram_tensor("w1", [H, H], dt.bfloat16, kind="ExternalInput")
    b1_d = nc.dram_tensor("b1", [P, 1], dt.float32, kind="ExternalInput")
    w2_d = nc.dram_tensor("w2", [H, C], dt.bfloat16, kind="ExternalInput")
    b2_d = nc.dram_tensor("b2c", [C, 1], dt.float32, kind="ExternalInput")
    out_o = nc.dram_tensor("out_o", [C, NPC], dt.bfloat16,
                           kind="ExternalOutput")

    MB = 4                                          # blocks per MLP group

    with tile.TileContext(nc) as tc:
        with (
            tc.tile_pool(name="const", bufs=1) as cp,
            tc.tile_pool(name="ein", bufs=2) as eip,
            tc.tile_pool(name="ework", bufs=2) as ewp,
            tc.tile_pool(name="lead", bufs=8) as ldp,
            tc.tile_pool(name="rows", bufs=8) as rp_,
            tc.tile_pool(name="win", bufs=max(len(mwin), 1)) as wnp,
            tc.tile_pool(name="gms", bufs=4) as gmp,
            tc.tile_pool(name="mw", bufs=3) as mwp,
            tc.tile_pool(name="mst", bufs=2) as mp,
            tc.tile_pool(name="ost", bufs=2) as stp,
            tc.tile_pool(name="ostm", bufs=2) as stp2,
            tc.tile_pool(name="tr_ps", bufs=2, space="PSUM") as tr_pp,
            tc.tile_pool(name="sel_ps", bufs=3, space="PSUM") as sel_pp,
            tc.tile_pool(name="mlp_ps", bufs=2, space="PSUM") as mlp_pp,
        ):
            w1_t = cp.tile([H, H], dt.bfloat16)
            nc.scalar.dma_start(w1_t[:], w1_d[:, :])
            b1_t = cp.tile([P, 1], dt.float32)
            nc.scalar.dma_start(b1_t[:], b1_d[:, :])
            w2_t = cp.tile([H, C], dt.bfloat16)
            nc.scalar.dma_start(w2_t[:], w2_d[:, :])
            b2_t = cp.tile([C, 1], dt.float32)
            nc.scalar.dma_start(b2_t[:], b2_d[:, :])
            ident_f = cp.tile([P, P], dt.bfloat16)
            make_identity(nc, ident_f[:])
            melog_t = cp.tile([P, MSC1], dt.float32)
            if MSCOLS > 0:
                nc.scalar.dma_start(melog_t[:], melog_d[:, :])

            def emit_mlp(meanT_stage, nmb, stage_out, g0):
                hpre_ps = mlp_pp.tile([P, MB * H], dt.float32,
                                      space="PSUM", tag="mlp")
                nc.tensor.matmul(out=hpre_ps[:, : nmb * H], lhsT=w1_t[:],
                                 rhs=meanT_stage[:, : nmb * P],
                                 start=True, stop=True)
                hT_stage = mp.tile([P, MB * H], dt.bfloat16, tag="hT")
                nc.scalar.activation(hT_stage[:, : nmb * H],
                                     hpre_ps[:, : nmb * H],
                                     mybir.ActivationFunctionType.Gelu,
                                     bias=b1_t[:, :1])
                rep_ps = mlp_pp.tile([P, MB * P], dt.float32,
                                     space="PSUM", tag="mlp")
                nc.tensor.matmul(out=rep_ps[:, : nmb * P], lhsT=w2_t[:],
                                 rhs=hT_stage[:, : nmb * H],
                                 start=True, stop=True)
                nc.scalar.activation(
                    stage_out[:, g0 * P: (g0 + nmb) * P],
                    rep_ps[:, : nmb * P],
                    mybir.ActivationFunctionType.Identity,
                    bias=b2_t[:, :1])

            # phase 1: indirect-side elections (DVE) -> per-chunk leadi
            leadis = []
            for (b0, ngb, Wc, coff) in chunks:
                n = ngb * Wc
                ep = eip.tile([P, CH * WMAX], dt.float32, tag="ep")
                nc.sync.dma_start(ep[:, :n], ep_d[:, coff: coff + n])
                epv = _view3(ep, ngb, Wc)
                sp1 = eip.tile([P, CH * WMAX], dt.float32, tag="sp1")
                nc.sync.dma_start(sp1[:, :n], sp1_d[:, coff: coff + n])
                sp1v = _view3(sp1, ngb, Wc)

                sm = ewp.tile([P, CH], dt.float32, tag="sm")
                nc.vector.reduce_max(out=sm[:, :ngb], in_=epv,
                                     axis=mybir.AxisListType.X)
                mask = ewp.tile([P, CH * WMAX], dt.float32, tag="mask")
                maskv = _view3(mask, ngb, Wc)
                a = sm[:]
                sm_b = bass.AP(a.tensor, a.offset,
                               [a.ap[0], [1, ngb], [0, Wc]])
                nc.vector.tensor_tensor(out=maskv, in0=epv, in1=sm_b,
                                        op=mybir.AluOpType.is_equal)
                cand = ewp.tile([P, CH * WMAX], dt.float32, tag="cand")
                candv = _view3(cand, ngb, Wc)
                nc.vector.tensor_tensor(out=candv, in0=maskv, in1=sp1v,
                                        op=mybir.AluOpType.mult)
                lp1 = ewp.tile([P, CH], dt.float32, tag="lp1")
                nc.vector.reduce_max(out=lp1[:, :ngb], in_=candv,
                                     axis=mybir.AxisListType.X)
                leadf = ewp.tile([P, CH], dt.float32, tag="leadf")
                nc.vector.tensor_scalar(
                    out=leadf[:, :ngb], in0=lp1[:, :ngb], scalar1=-1.0,
                    scalar2=0.0,
                    op0=mybir.AluOpType.add, op1=mybir.AluOpType.max)
                leadi = ldp.tile([P, CH], dt.int32, tag="leadi")
                nc.vector.tensor_copy(leadi[:, :ngb], leadf[:, :ngb])
                leadis.append(leadi)

            # phase 2: issue all indirect gathers (GpSimd self-paces; they
            # drain after the mset stream quiesces)
            rows_l = []
            for ci, (b0, ngb, Wc, coff) in enumerate(chunks):
                leadi = leadis[ci]
                rows = rp_.tile([P, CH, H], dt.bfloat16, tag="rows")
                for j in range(ngb):
                    nc.gpsimd.indirect_dma_start(
                        out=rows[:, j, :],
                        out_offset=None,
                        in_=means_d[:, :],
                        in_offset=bass.IndirectOffsetOnAxis(
                            ap=leadi[:, j: j + 1], axis=0),
                    )
                rows_l.append(rows)

            # phase 3: masked-side winner masks (DVE)
            wins = []
            for (b0m, nblk, Kw, col0m) in mwin:
                nmc = nblk * Kw
                smx = ewp.tile([P, MSW], dt.float32, tag="smx")
                ml = melog_t[:, col0m: col0m + nmc]
                lv = bass.AP(ml.tensor, ml.offset,
                             [ml.ap[0], [Kw, nblk], [1, Kw]])
                nc.vector.reduce_max(out=smx[:, :nblk], in_=lv,
                                     axis=mybir.AxisListType.X)
                win = wnp.tile([P, MSW], dt.bfloat16, tag="win")
                winv = _view3(win, nblk, Kw)
                a = smx[:]
                smx_b = bass.AP(a.tensor, a.offset,
                                [a.ap[0], [1, nblk], [0, Kw]])
                nc.vector.tensor_tensor(out=winv, in0=lv, in1=smx_b,
                                        op=mybir.AluOpType.is_equal)
                wins.append(win)

            # phase 4: mset stream + select + MLP
            gp_elems, dve_elems = 0, 1
            for wi, (b0m, nblk, Kw, col0m) in enumerate(mwin):
                nmc = nblk * Kw
                G = gmp.tile([P, MSW * H], dt.bfloat16, tag="gms")
                nc.sync.dma_start(G[:, : nmc * H],
                                  mset_d[:, col0m * H: (col0m + nmc) * H])
                win = wins[wi]
                mw = mwp.tile([P, MSW * H], dt.bfloat16, tag="mw")
                ga = G[:, : nmc * H]
                gv = bass.AP(ga.tensor, ga.offset,
                             [ga.ap[0], [H, nmc], [1, H]])
                wa = win[:, : nmc]
                wv = bass.AP(wa.tensor, wa.offset,
                             [wa.ap[0], [1, nmc], [0, H]])
                ma = mw[:, : nmc * H]
                mv = bass.AP(ma.tensor, ma.offset,
                             [ma.ap[0], [H, nmc], [1, H]])
                if gp_elems * 2 < dve_elems:
                    eng = nc.gpsimd
                    gp_elems += nmc
                else:
                    eng = nc.vector
                    dve_elems += nmc
                eng.tensor_tensor(out=mv, in0=gv, in1=wv,
                                  op=mybir.AluOpType.mult)

                stage_out = stp2.tile([P, 12 * P], dt.bfloat16, tag="stm")
                meanT_stage = None
                for bi in range(nblk):
                    gj = bi % MB
                    if gj == 0:
                        meanT_stage = mp.tile([P, MB * P], dt.bfloat16,
                                              tag="meanT")
                    sel_ps = sel_pp.tile([P, 512], dt.float32, space="PSUM",
                                         tag="sel")
                    for j in range(Kw):
                        c = bi * Kw + j
                        nc.tensor.matmul(out=sel_ps[:, :P],
                                         lhsT=mw[:, c * H: (c + 1) * H],
                                         rhs=ident_f[:],
                                         start=(j == 0), stop=(j == Kw - 1))
                    nc.scalar.activation(
                        meanT_stage[:, gj * P: (gj + 1) * P], sel_ps[:, :P],
                        mybir.ActivationFunctionType.Copy)
                    if gj == MB - 1 or bi == nblk - 1:
                        emit_mlp(meanT_stage, gj + 1, stage_out, bi - gj)
                nc.scalar.dma_start(out_o[:, b0m * P: (b0m + nblk) * P],
                                    stage_out[:, : nblk * P])

            # phase 5: indirect-side MLP
            for ci, (b0, ngb, Wc, coff) in enumerate(chunks):
                rows = rows_l[ci]
                stage_out = stp.tile([P, CH * P], dt.bfloat16, tag="st")
                for g0 in range(0, ngb, MB):
                    nmb = min(MB, ngb - g0)
                    meanT_stage = mp.tile([P, MB * P], dt.bfloat16,
                                          tag="meanT")
                    for j in range(nmb):
                        meanT_ps = tr_pp.tile([P, P], dt.bfloat16,
                                              space="PSUM", tag="tr")
                        nc.tensor.transpose(meanT_ps[:], rows[:, g0 + j, :],
                                            ident_f[:])
                        nc.scalar.activation(
                            meanT_stage[:, j * P: (j + 1) * P], meanT_ps[:],
                            mybir.ActivationFunctionType.Copy)
                    emit_mlp(meanT_stage, nmb, stage_out, g0)
                nc.scalar.dma_start(out_o[:, b0 * P: (b0 + ngb) * P],
                                    stage_out[:, : ngb * P])
    nc.compile()
    return nc


# ---------------------------------------------------------------------------

_CACHE = {}


def _get(key, fn):
    if key not in _CACHE:
        _CACHE[key] = fn()
    return _CACHE[key]


def kernel(x, edge_index, w_lead, b_lead, w1, b1, w2, b2):
    x = np.asarray(x, np.float32)
    assert x.shape == (N_NODES, H)

    ekey = hashlib.md5(np.asarray(edge_index).tobytes()).hexdigest()
    pp = _get(("pp", ekey), lambda: _preprocess(edge_index))

    K = pp["K"]
    inv = pp["inv"]
    inv_flat = inv.reshape(NCORES, NPC)

    xbf_ext = np.zeros((N_NODES + 1, H), bf16)
    xbf_ext[:N_NODES] = x.astype(bf16)
    xpad = np.zeros((NPAD, H), np.float32)
    xpad[:N_NODES] = x

    wrepc = np.ascontiguousarray(
        np.asarray(w_lead, np.float32).reshape(H, 1))
    blead = np.full((1, 1), np.float32(b_lead), np.float32)
    w1f = np.ascontiguousarray(np.asarray(w1, np.float32).astype(bf16))
    b1c = np.ascontiguousarray(np.asarray(b1, np.float32).reshape(H, 1))
    w2f = np.ascontiguousarray(np.asarray(w2, np.float32).astype(bf16))
    b2c = np.ascontiguousarray(np.asarray(b2, np.float32).reshape(C, 1))

    # ---- launch 1 ----------------------------------------------------------
    nc1 = _get(("l1", ekey), lambda: _build_l1(pp))
    in_maps = []
    for k in range(NCORES):
        # ET[k]: [P, K*H] partition-major slot table (slot = c*128+p)
        et = xbf_ext[pp["srcidx"][k]].reshape(K, P, H).transpose(1, 0, 2)
        et = np.ascontiguousarray(et).reshape(P, K * H)
        # xfT: [H, NPC] owned-node features transposed (f32, for logits)
        xf = np.where((inv[k] >= 0)[:, :, None],
                      xpad[np.maximum(inv[k], 0)], 0.0)  # [BPC, P, H]
        xfT = np.ascontiguousarray(
            xf.reshape(NPC, H).T.astype(np.float32))
        in_maps.append({
            "et": et,
            "xfT": xfT,
            "recip": pp["recip"][k],
            "wrepc": wrepc,
            "blead": blead,
        })
    r1 = run_bass_kernel_spmd(nc1, in_maps, core_ids=CORES)

    logits_full = np.zeros(NPAD, np.float32)
    means_full = np.zeros((NPAD, H), bf16)
    for k in range(NCORES):
        lg = r1.results[k]["logits_o"].reshape(NPC)     # [b*128+p]
        mn = r1.results[k]["means_o"].reshape(P, BPC, H)
        mf = inv_flat[k] >= 0
        ids = inv_flat[k][mf]
        logits_full[ids] = lg[mf]
        m = inv[k] >= 0                                 # [BPC, P]
        means_full[inv[k][m]] = mn.transpose(1, 0, 2)[m]

    # ---- launch 2: election + mean gather + MLP ----------------------------
    nc2 = _get(("l2", ekey), lambda: _build_l2(pp))
    es = pp["elog_src"]
    msrc = pp["msrc"]
    MSCOLS = pp["MSCOLS"]
    logits_ext = np.full(NPAD + 1, NEG, np.float32)
    logits_ext[:NPAD] = logits_full
    in_maps2 = []
    zpad = np.zeros((P, 1), np.float32)
    for k in range(NCORES):
        if pp["SWR"] > 0:
            ep = np.ascontiguousarray(
                np.where(es[k] >= 0, logits_full[np.maximum(es[k], 0)],
                         NEG).astype(np.float32))
            sp1k = pp["srcp1"][k]
        else:
            ep, sp1k = zpad, zpad
        if MSCOLS > 0:
            melog = np.where(msrc[k] < N_NODES, logits_ext[msrc[k]],
                             NEG).astype(np.float32)
            mset = means_full[msrc[k]].reshape(P, MSCOLS * H)
        else:
            melog = zpad
            mset = np.zeros((P, H), bf16)
        in_maps2.append({
            "epad": ep,
            "srcp1": sp1k,
            "meansfull": means_full,
            "melog": np.ascontiguousarray(melog),
            "mset": np.ascontiguousarray(mset),
            "w1": w1f,
            "b1": b1c,
            "w2": w2f,
            "b2c": b2c,
        })
    r2 = run_bass_kernel_spmd(nc2, in_maps2, core_ids=CORES)

    out = np.zeros((N_NODES, C), np.float32)
    for k in range(NCORES):
        g = r2.results[k]["out_o"].reshape(C, BPC, P).astype(np.float32)
        node_rows = g.transpose(1, 2, 0).reshape(NPC, C)
        m = inv_flat[k] >= 0
        out[inv_flat[k][m]] = node_rows[m]
    return out
